# revision 42
# baseline (speedup 1.0000x reference)
"""MoCo loss (InfoNCE over a 65536-entry queue + proto-NCE over 50000
k-means centroids) on 8 Trainium2 NeuronCores.

fp8e4m3 operands with DoubleRowSwInterleave matmuls (2 contraction
subtiles per instruction; the stationary operand is pre-interleaved on
the host -- A/B k-layer pairs per column, columns reversed -- so the
weight load reads contiguously).  Tables are scaled by 16 per operand
(dots come out x256) and quantized to e4m3 on the host.

Per core (tables sharded by row, Z_q replicated):

  part 2 (centroid shard, zq stationary / centroids moving):
    - s2 = Z_q @ c_shard.T  (PE, fp32 acc), cast fp8 (DVE/ACT), export
      (argmax + exclusion gather on the host)
  part 1 (queue shard, queue stationary / zq moving):
    - s1 = q_shard @ Z_q.T                 (PE)
    - exp(s1/T) in fp8  (no shift; values in [e^-4, e^4])   (ACT)
    - per-queue-row max of the exp tiles -> rm export       (DVE)
    - ones DoubleRow matmul partition sum -> per-batch partial
      sum-of-exp                                            (PE)

The kernel streams ~7.3 MiB of table reads per core on both HWDGE
rings (sync + scalar); part-1 tiles and part-2 chunks are emitted
interleaved in DMA-arrival order so the PE never starves; part-2's
last chunks are the final PE work so the end-of-kernel chain is just
cast -> one merged export (s2 tail + rm + fin share one DRAM tensor
with fat 6.5 KiB lines).

The host combines per-core partials (logsumexp merge, global argmax,
exclusion gather + 513-wide softmax).  Both outputs stay exact despite
fp8 noise: the host computes s0 = queue @ Z_q[0] itself and re-checks
every queue row within MARGIN of the device rowmax -- and every
centroid column within MARGIN2 of its row max -- with full-precision
dot products (fp8 dot error is < 0.01).
"""

import os
import numpy as np
import ml_dtypes

B, C = 256, 512
QUEUE, NCL, NNEG = 65536, 50000, 512
INFO_TEMP = 0.07
PROTO_FACTOR = 0.5
NCORES = 8
QSH = QUEUE // NCORES          # 8192 queue rows per core
CSH = NCL // NCORES            # 6250 centroid rows per core
CSH_PAD = 6272                 # 14 * 448
CCH = 14                       # s2 matmul chunks
CW = CSH_PAD // CCH            # 448
KSUB = C // 128                # 4 contraction subtiles
KPAIR = KSUB // 2              # 2 DoubleRow pairs
NBT = 16                       # part-1 big tiles (512 queue rows each)
QCHUNK = 4                     # qT DMA chunks (8 KiB per-partition lines)
JW = QSH // QCHUNK             # 2048
JSUB = JW // 128               # 16 queue subtiles per chunk
FP8_SCALE = 16.0               # per-operand scale; dots come out x256
DOT_SCALE = FP8_SCALE * FP8_SCALE
MARGIN = 0.05                  # host re-check threshold (unscaled units)
MARGIN2 = 0.045                # part-2 argmax re-check threshold

# merged export regions (bytes per partition inside the SBUF out tile).
# DRAM side is THREE separate contiguous tensors (outA/outB/outC) --
# strided DRAM writes run ~4x slower than contiguous ones.
S2_BYTES = CCH * 2 * CW        # 12544
RM_OFF = S2_BYTES              # [128, NBT*4] fp32 = 256 B
FIN_OFF = RM_OFF + NBT * 4 * 4  # [128, 256] fp32 = 1024 B
OUT_BYTES = FIN_OFF + B * 4    # 13824
# out_sb region order: [ch2..ch13 | ch0 | ch1 | rm | fin] so the final
# export (outC) carries only late-finishing small pieces
OUTA_END = 6 * 2 * CW          # chunks 2-7   (5376 B lines)
OUTB_END = 11 * 2 * CW         # chunks 8-12  (4480 B lines)
# outC = ch13 + ch0 + ch1 + rm + fin (3968 B lines, the final export)

# s2 cast engines: head chunks + late-tail evens on DVE, the rest on ACT
# (ACT covers the early tail while DVE finishes the rm reduces)
DVE_CAST_CH = frozenset((0, 1, 2, 3, 4, 5, 6, 8, 10, 12))
# rm reduce groups: big groups while exps stream; the last 4 tiles reduce
# per half-tile so the post-exp tail chain is one 512-elem reduce
RM_GROUPS = ((0, 4), (4, 8), (8, 10), (10, 12))
RM_HALF_FROM = 12

_CACHE = {}

# exec time of the last device run (ns), populated when tracing is on
last_exec_time_ns = None


def _build():
    import concourse.bass as bass
    import concourse.tile as tile
    from concourse import bacc, mybir

    dt = mybir.dt
    DRI = mybir.MatmulPerfMode.DoubleRowSwInterleave
    nc = bacc.Bacc(
        "TRN2", target_bir_lowering=False, debug=False, num_devices=NCORES
    )

    # ---- DRAM I/O (all partition-major so every DMA is [128, N] flat) ----
    zq_d = nc.dram_tensor("zq", [128, 8, 256], dt.float8e4, kind="ExternalInput").ap()
    qt_d = [
        nc.dram_tensor(
            f"qt{h}", [128, JSUB, KPAIR, 256], dt.float8e4, kind="ExternalInput"
        ).ap()
        for h in range(QCHUNK)
    ]
    # head = cTb0 | cTb1 | cTa0a in one transfer (lands in one burst)
    head_d = nc.dram_tensor(
        "head", [128, 2 * KSUB * CW + KSUB * 2 * CW], dt.float8e4,
        kind="ExternalInput",
    ).ap()
    cTa0b_d = nc.dram_tensor(
        "cTa0b", [128, KSUB, 2 * CW], dt.float8e4, kind="ExternalInput"
    ).ap()
    cTa1_d = nc.dram_tensor(
        "cTa1", [128, KSUB, 4 * CW], dt.float8e4, kind="ExternalInput"
    ).ap()
    cTa2_d = nc.dram_tensor(
        "cTa2", [128, KSUB, 4 * CW], dt.float8e4, kind="ExternalInput"
    ).ap()

    # merged output, three contiguous DRAM tensors:
    # outA = s2 chunks 0-5, outB = chunks 6-11, outC = chunks 12-13|rm|fin
    outA_d = nc.dram_tensor(
        "outA", [128, OUTA_END], dt.float8e4, kind="ExternalOutput"
    ).ap()
    outB_d = nc.dram_tensor(
        "outB", [128, OUTB_END - OUTA_END], dt.float8e4, kind="ExternalOutput"
    ).ap()
    outC_d = nc.dram_tensor(
        "outC", [128, OUT_BYTES - OUTB_END], dt.float8e4, kind="ExternalOutput"
    ).ap()

    with tile.TileContext(nc) as tc:
        with (
            tc.tile_pool(name="const", bufs=1) as cpool,
            tc.tile_pool(name="ps1", bufs=2, space="PSUM") as ps1,
            tc.tile_pool(name="psum1s", bufs=1, space="PSUM") as ps1s,
            tc.tile_pool(name="ps2", bufs=3, space="PSUM") as ps2,
        ):
            # ---- resident SBUF tensors ----
            zq_sb = cpool.tile([128, 8, 256], dt.float8e4)
            head_sb = cpool.tile([128, 4 * KSUB * CW], dt.float8e4, tag="head")
            cTb_sb = [
                head_sb[:, i * KSUB * CW : (i + 1) * KSUB * CW].rearrange(
                    "p (k w) -> p k w", k=KSUB, w=CW
                )
                for i in range(2)
            ]
            cTa0a_sb = head_sb[:, 2 * KSUB * CW :].rearrange(
                "p (k w) -> p k w", k=KSUB, w=2 * CW
            )
            cTa0b_sb = cpool.tile([128, KSUB, 2 * CW], dt.float8e4, tag="cTa0b")
            cTa1_sb = cpool.tile([128, KSUB, 4 * CW], dt.float8e4, tag="cTa1")
            cTa2_sb = cpool.tile([128, KSUB, 4 * CW], dt.float8e4, tag="cTa2")
            qt_sb = [
                cpool.tile(
                    [128, JSUB, KPAIR, 256], dt.float8e4, name=f"qt{h}", tag=f"qt{h}"
                )
                for h in range(QCHUNK)
            ]

            # ALL input DMAs on the sync HWDGE ring: it has strict priority
            # over the scalar ring, so one FIFO in exact consumption order
            # gives full bandwidth to the critical stream and exact arrival
            # order.  Exports ride gpsimd (SWDGE) except the final outC.
            nc.sync.dma_start(zq_sb[:], zq_d[:])
            nc.sync.dma_start(head_sb[:], head_d[:])
            nc.sync.dma_start(qt_sb[0][:], qt_d[0][:])
            nc.sync.dma_start(cTa0b_sb[:], cTa0b_d[:])
            nc.sync.dma_start(qt_sb[1][:], qt_d[1][:])
            nc.sync.dma_start(qt_sb[2][:], qt_d[2][:])
            nc.sync.dma_start(qt_sb[3][:], qt_d[3][:])
            nc.sync.dma_start(cTa1_sb[:], cTa1_d[:])
            nc.sync.dma_start(cTa2_sb[:], cTa2_d[:])

            # interleave/column-reversal of all-ones is all-ones
            ones_sb = cpool.tile([128, 256], dt.float8e4)
            nc.vector.memset(ones_sb[:], 1.0)

            # merged export tile + typed views
            out_sb = cpool.tile([128, OUT_BYTES], dt.float8e4)
            s2_v = out_sb[:, 0:S2_BYTES].rearrange(
                "p (c b w) -> p c b w", c=CCH, b=2, w=CW
            )
            rm_v = out_sb[:, RM_OFF:FIN_OFF].bitcast(dt.float32)   # [128, 64]
            fin_v = out_sb[:, FIN_OFF:OUT_BYTES].bitcast(dt.float32)  # [128, 256]

            exp_all = cpool.tile([128, NBT, 4, B], dt.float8e4)
            exp_tiles = [exp_all[:, t] for t in range(NBT)]

            fin_sb = None  # ACT writes fin via fin_v

            # ---- emission helpers ----
            def emit_chunk(ch):
                """part-2 chunk: s2[:, ch] = Z_q @ c_chunk.T, cast to fp8."""
                if ch < 2:
                    cmov, w = cTb_sb[ch], 0
                elif ch < 4:
                    cmov, w = cTa0a_sb, ch - 2
                elif ch < 6:
                    cmov, w = cTa0b_sb, ch - 4
                elif ch < 10:
                    cmov, w = cTa1_sb, ch - 6
                else:
                    cmov, w = cTa2_sb, ch - 10
                for bt in range(2):
                    s2_ps = ps2.tile([128, CW], dt.float32, tag="s2")
                    for kp in range(KPAIR):
                        nc.tensor.matmul(
                            s2_ps[:],
                            zq_sb[:, 4 + 2 * kp + bt, :],
                            cmov[:, 2 * kp : 2 * kp + 2, w * CW : (w + 1) * CW],
                            start=(kp == 0),
                            stop=(kp == KPAIR - 1),
                            perf_mode=DRI,
                        )
                    slot = ch - 2 if ch >= 2 else 12 + ch
                    if ch in DVE_CAST_CH:
                        nc.vector.tensor_copy(s2_v[:, slot, bt, :], s2_ps[:])
                    else:
                        nc.scalar.copy(s2_v[:, slot, bt, :], s2_ps[:])

            rm_after = {b - 1: (a, b) for a, b in RM_GROUPS}

            def emit_tile(t, p1s_ps):
                """part-1 tile: 512 queue rows -> exp fp8; lagged ones-sum;
                rm reduce at group boundaries (per half-tile near the end)."""
                s1_ps = ps1.tile([128, 4, B], dt.float32, tag="s1")
                for q in range(4):
                    jt = t * 4 + q
                    h, jl = divmod(jt, JSUB)
                    for kp in range(KPAIR):
                        nc.tensor.matmul(
                            s1_ps[:, q, :],
                            qt_sb[h][:, jl, kp, :],
                            zq_sb[:, 2 * kp : 2 * kp + 2, :],
                            start=(kp == 0),
                            stop=(kp == KPAIR - 1),
                            perf_mode=DRI,
                        )
                nc.scalar.activation(
                    exp_tiles[t][:],
                    s1_ps[:],
                    mybir.ActivationFunctionType.Exp,
                    scale=1.0 / (DOT_SCALE * INFO_TEMP),
                )
                if t > 1:
                    # two tiles behind: ACT's exp has a full tile of slack
                    for g in range(2):
                        nc.tensor.matmul(
                            p1s_ps[:],
                            ones_sb[:],
                            exp_tiles[t - 2][:, 2 * g : 2 * g + 2, :],
                            start=(t == 2 and g == 0),
                            stop=False,
                            perf_mode=DRI,
                        )
                if t >= RM_HALF_FROM:
                    for hh in range(2):
                        nc.vector.tensor_reduce(
                            rm_v[:, t * 4 + 2 * hh : t * 4 + 2 * hh + 2],
                            exp_all[:, t, 2 * hh : 2 * hh + 2],
                            axis=mybir.AxisListType.X,
                            op=mybir.AluOpType.max,
                        )
                elif t in rm_after:
                    a, b = rm_after[t]
                    nc.vector.tensor_reduce(
                        rm_v[:, a * 4 : b * 4],
                        exp_all[:, a:b],
                        axis=mybir.AxisListType.X,
                        op=mybir.AluOpType.max,
                    )

            # ---- emission: warmup, head chunks, part-1 backbone, tail ----
            p1s_ps = ps1s.tile([128, B], dt.float32)   # sum-of-exp accum

            # HAM warmup: ~3.4us of dummy matmuls on zq (the first input to
            # land) so the real work runs at 2.4 GHz from the start.  The
            # results are never read; the psum pool recycles the banks.
            for i in range(16):
                w_ps = ps2.tile([128, CW], dt.float32, tag="s2")
                nc.tensor.matmul(
                    w_ps[:, 0:B],
                    zq_sb[:, 4, :],
                    zq_sb[:, 0:2, :],
                    start=True,
                    stop=True,
                    perf_mode=DRI,
                )

            for ch in range(4):
                emit_chunk(ch)
            for t in range(4):
                emit_tile(t, p1s_ps)
            emit_chunk(4)
            emit_chunk(5)
            for t in range(4, 10):
                emit_tile(t, p1s_ps)
            # tail chunks interleave with the last tiles so their casts
            # (and exports) complete with the compute, not after it
            emit_chunk(6)
            emit_tile(10, p1s_ps)
            emit_chunk(7)
            # outA: chunks 2-7 (overlaps the remaining compute)
            nc.gpsimd.dma_start(outA_d[:], out_sb[:, 0:OUTA_END])
            emit_tile(11, p1s_ps)
            emit_chunk(8)
            emit_tile(12, p1s_ps)
            emit_chunk(9)
            emit_tile(13, p1s_ps)
            emit_chunk(10)
            emit_tile(14, p1s_ps)
            emit_chunk(11)
            emit_tile(15, p1s_ps)
            emit_chunk(12)
            # flush the lagged ones-sum (tiles 14, 15)
            for t in (NBT - 2, NBT - 1):
                for g in range(2):
                    nc.tensor.matmul(
                        p1s_ps[:],
                        ones_sb[:],
                        exp_tiles[t][:, 2 * g : 2 * g + 2, :],
                        start=False,
                        stop=(t == NBT - 1 and g == 1),
                        perf_mode=DRI,
                    )
            nc.scalar.copy(fin_v[:], p1s_ps[:])
            nc.gpsimd.dma_start(outB_d[:], out_sb[:, OUTA_END:OUTB_END])
            emit_chunk(13)
            # final short export: ch13 + ch0-1 + rm + fin (3968 B lines)
            nc.sync.dma_start(outC_d[:], out_sb[:, OUTB_END:OUT_BYTES])

    nc.compile()
    return nc


def _get_nc():
    if "nc" not in _CACHE:
        _CACHE["nc"] = _build()
    return _CACHE["nc"]


def _to_fp8(x):
    return (x * FP8_SCALE).astype(ml_dtypes.float8_e4m3fn)


def _interleave(A, B):
    """SwInterleave weight layout: mem[p, 2*jj+i] = layer_i[p, 127-jj].
    A, B: [..., 128, 128] (partition, column)."""
    return np.stack([A[..., ::-1], B[..., ::-1]], axis=-1).reshape(
        *A.shape[:-1], 256
    )


def _prep_inputs(Z_q, queue, centroids):
    """Host-side shard prep: x16 scale + e4m3 quantization + transpose to
    [C, rows], then partition-major chunk layouts so each DMA is a flat
    [128, N].  Stationary operands are pre-interleaved for SwInterleave."""
    zqT8 = _to_fp8(Z_q).T                            # [512, 256]
    zqT = zqT8.reshape(KSUB, 128, B).transpose(1, 0, 2)  # [128, KSUB, B]
    # part-2 stationary: [128, kp*2+bt, 256] interleaved
    zz = zqT8.reshape(KPAIR, 2, 128, 2, 128)         # [kp, i, p, bt, col]
    zqTi = (
        _interleave(zz[:, 0], zz[:, 1])
        .transpose(1, 0, 2, 3)
        .reshape(128, KSUB, 256)
    )
    zq = np.ascontiguousarray(np.concatenate([zqT, zqTi], axis=1))  # [128, 8, 256]

    qT = np.ascontiguousarray(_to_fp8(queue).T)      # [512, 65536]
    cT = np.ascontiguousarray(_to_fp8(centroids).T)  # [512, 50000]

    in_maps = []
    for i in range(NCORES):
        q_sh = qT[:, i * QSH : (i + 1) * QSH]        # [512, 8192]
        # [kp, i, p, h, jl, col]
        qq = q_sh.reshape(KPAIR, 2, 128, QCHUNK, JSUB, 128)
        q_sh = np.ascontiguousarray(
            _interleave(qq[:, 0], qq[:, 1]).transpose(2, 1, 3, 0, 4)
        )  # [QCHUNK, 128, JSUB, KPAIR, 256]
        qt_chunks = {f"qt{h}": q_sh[h] for h in range(QCHUNK)}
        c_sh = np.zeros((C, CSH_PAD), ml_dtypes.float8_e4m3fn)
        c_sh[:, :CSH] = cT[:, i * CSH : (i + 1) * CSH]
        # cTb0/cTb1 = first 2 matmul chunks (small, land first); cTa = rest
        def cpack(lo, hi):
            return np.ascontiguousarray(
                c_sh[:, lo * CW : hi * CW]
                .reshape(KSUB, 128, (hi - lo) * CW)
                .transpose(1, 0, 2)
            )

        head = np.ascontiguousarray(
            np.concatenate(
                [cpack(0, 1).reshape(128, -1), cpack(1, 2).reshape(128, -1),
                 cpack(2, 4).reshape(128, -1)],
                axis=1,
            )
        )  # [128, 7168]: cTb0 | cTb1 | cTa0a per partition
        in_maps.append({"zq": zq, **qt_chunks, "head": head,
                        "cTa0b": cpack(4, 6),
                        "cTa1": cpack(6, 10), "cTa2": cpack(10, 14)})
    return in_maps


def kernel(Z_q, Z_k, queue, centroids, kmeans_temp, neg_raw):
    global last_exec_time_ns
    from concourse.bass_utils import run_bass_kernel_spmd

    nc = _get_nc()
    in_maps = _prep_inputs(Z_q, queue, centroids)

    trace = bool(int(os.environ.get("MOCO_BASS_TRACE", "0")))
    out = run_bass_kernel_spmd(nc, in_maps, core_ids=list(range(NCORES)), trace=trace)
    last_exec_time_ns = out.exec_time_ns
    res = out.results

    # decode the merged export per core
    def regions(r):
        raw = np.concatenate([r["outA"], r["outB"], r["outC"]], axis=1)
        sr = raw[:, :S2_BYTES].astype(np.float32).reshape(128, CCH, 2, CW)
        # slot order is [ch2..ch13, ch0, ch1] -- restore chunk order
        s2 = np.concatenate([sr[:, 12:14], sr[:, 0:12]], axis=1)
        rm = np.ascontiguousarray(raw[:, RM_OFF:FIN_OFF]).view(np.float32)
        fin = np.ascontiguousarray(raw[:, FIN_OFF:]).view(np.float32)
        return s2, rm, fin

    decoded = [regions(r) for r in res]

    # ---- host combine (tiny) ----
    lp = (Z_q.astype(np.float64) * Z_k.astype(np.float64)).sum(axis=1)  # l_pos
    lp_t = lp / INFO_TEMP

    # part-1 loss: logsumexp over [l_pos | l_neg]/T per batch row.
    # Device partials are unshifted sums of e^{s/T} (|s/T| <= ~4).
    S = np.zeros(B, np.float64)
    for _, _, fin in decoded:
        S += fin[0].astype(np.float64)
    S += np.exp(lp_t)
    lse1 = np.log(S)
    loss1 = np.mean(lse1 - lp_t)

    # accuracy: exact despite fp8 scores.  The device reduces the fp8 exp
    # tiles over the batch axis (rm = max_b exp(s/T), fp32-exact); every
    # row with margin < MARGIN is re-checked on the host in full precision.
    rm_full = np.empty(QUEUE, np.float64)
    for i, (_, rm, _) in enumerate(decoded):
        # rm[p, jt] -> queue row j = jt*128 + p
        rm_full[i * QSH : (i + 1) * QSH] = (
            np.log(rm.astype(np.float64).T.reshape(-1)) * INFO_TEMP
        )

    # s0 computed exactly on the host (33 MFLOP) -- only rm comes from
    # the device, so the margin test has one noisy side instead of two
    s0_full = queue.astype(np.float64) @ Z_q[0].astype(np.float64)
    cand = (rm_full - s0_full) < MARGIN
    cols = np.nonzero(cand)[0]
    sub = Z_q.astype(np.float64) @ queue[cols].astype(np.float64).T  # [B, ncand]
    count = float((sub[0] >= sub.max(axis=0)).sum())
    count += float(lp[0] >= lp.max())
    accuracy = count / (1 + QUEUE)

    # part-2: global argmax over centroids (== argmin of ||c||^2 - 2 s).
    # s2 arrives in fp8; the argmax (and the positive logit) is resolved
    # exactly by re-checking every near-max column in full precision.
    s2_full = np.empty((B, NCL), np.float32)
    for i, (s2, _, _) in enumerate(decoded):
        sh = s2.transpose(2, 0, 1, 3).reshape(B, CSH_PAD)
        s2_full[:, i * CSH : (i + 1) * CSH] = sh[:, :CSH]
    s2_full /= DOT_SCALE

    kt = kmeans_temp.astype(np.float64)
    Zq64 = Z_q.astype(np.float64)
    ce64 = centroids.astype(np.float64)
    mx = s2_full.max(axis=1)
    I = np.empty(B, np.int64)
    pl_pos = np.empty(B)
    for b in range(B):
        cnd = np.nonzero(s2_full[b] >= mx[b] - MARGIN2)[0]
        ex = ce64[cnd] @ Zq64[b]
        k = int(np.argmax(ex))
        I[b] = cnd[k]
        pl_pos[b] = ex[k] / kt[cnd[k]]

    neg_idx = neg_raw + (neg_raw >= I[:, None]).astype(neg_raw.dtype)
    pl_neg = (
        np.take_along_axis(s2_full, neg_idx, axis=1).astype(np.float64)
        / kt[neg_idx]
    )
    plogits = np.concatenate([pl_pos[:, None], pl_neg], axis=1)
    m = plogits.max(axis=1)
    plse = np.log(np.exp(plogits - m[:, None]).sum(axis=1)) + m
    ploss = np.mean(plse - pl_pos)

    loss = loss1 + PROTO_FACTOR * ploss
    return np.float32(loss), np.float32(accuracy)


# revision 43
# speedup vs baseline: 1.0191x; 1.0191x over previous
"""MoCo loss (InfoNCE over a 65536-entry queue + proto-NCE over 50000
k-means centroids) on 8 Trainium2 NeuronCores.

fp8e4m3 operands with DoubleRowSwInterleave matmuls (2 contraction
subtiles per instruction; the stationary operand is pre-interleaved on
the host -- A/B k-layer pairs per column, columns reversed -- so the
weight load reads contiguously).  Tables are scaled by 16 per operand
(dots come out x256) and quantized to e4m3 on the host.

Per core (tables sharded by row, Z_q replicated):

  part 2 (centroid shard, zq stationary / centroids moving):
    - s2 = Z_q @ c_shard.T  (PE, fp32 acc), cast fp8 (DVE/ACT), export
      (argmax + exclusion gather on the host)
  part 1 (queue shard, queue stationary / zq moving):
    - s1 = q_shard @ Z_q.T                 (PE)
    - exp(s1/T) in fp8  (no shift; values in [e^-4, e^4])   (ACT)
    - per-queue-row max of the exp tiles -> rm export       (DVE)
    - ones DoubleRow matmul partition sum -> per-batch partial
      sum-of-exp                                            (PE)

The kernel streams ~7.3 MiB of table reads per core on both HWDGE
rings (sync + scalar); part-1 tiles and part-2 chunks are emitted
interleaved in DMA-arrival order so the PE never starves; part-2's
last chunks are the final PE work so the end-of-kernel chain is just
cast -> one merged export (s2 tail + rm + fin share one DRAM tensor
with fat 6.5 KiB lines).

The host combines per-core partials (logsumexp merge, global argmax,
exclusion gather + 513-wide softmax).  Both outputs stay exact despite
fp8 noise: the host computes s0 = queue @ Z_q[0] itself and re-checks
every queue row within MARGIN of the device rowmax -- and every
centroid column within MARGIN2 of its row max -- with full-precision
dot products (fp8 dot error is < 0.01).
"""

import os
import numpy as np
import ml_dtypes

B, C = 256, 512
QUEUE, NCL, NNEG = 65536, 50000, 512
INFO_TEMP = 0.07
PROTO_FACTOR = 0.5
NCORES = 8
QSH = QUEUE // NCORES          # 8192 queue rows per core
CSH = NCL // NCORES            # 6250 centroid rows per core
CSH_PAD = 6272                 # 14 * 448
CCH = 14                       # s2 matmul chunks
CW = CSH_PAD // CCH            # 448
KSUB = C // 128                # 4 contraction subtiles
KPAIR = KSUB // 2              # 2 DoubleRow pairs
NBT = 16                       # part-1 big tiles (512 queue rows each)
QCHUNK = 4                     # qT DMA chunks (8 KiB per-partition lines)
JW = QSH // QCHUNK             # 2048
JSUB = JW // 128               # 16 queue subtiles per chunk
FP8_SCALE = 16.0               # per-operand scale; dots come out x256
DOT_SCALE = FP8_SCALE * FP8_SCALE
MARGIN = 0.05                  # host re-check threshold (unscaled units)
MARGIN2 = 0.045                # part-2 argmax re-check threshold

# merged export regions (bytes per partition inside the SBUF out tile).
# DRAM side is THREE separate contiguous tensors (outA/outB/outC) --
# strided DRAM writes run ~4x slower than contiguous ones.
S2_BYTES = CCH * 2 * CW        # 12544
RM_OFF = S2_BYTES              # [128, NBT*4] fp32 = 256 B
FIN_OFF = RM_OFF + NBT * 4 * 4  # [128, 256] fp32 = 1024 B
OUT_BYTES = FIN_OFF + B * 4    # 13824
OUTA_END = 6 * 2 * CW          # chunks 0-5   (5376 B lines)
OUTB_END = 10 * 2 * CW         # chunks 6-9   (3584 B lines)
# outC = chunks 10-13 + rm + fin (4864 B lines, the final export)

# s2 cast engines: head chunks + late-tail evens on DVE, the rest on ACT
# (ACT covers the early tail while DVE finishes the rm reduces)
DVE_CAST_CH = frozenset((0, 1, 2, 3, 4, 5, 10, 12))
# rm reduce groups: big groups while exps stream; the last 4 tiles reduce
# per half-tile so the post-exp tail chain is one 512-elem reduce
RM_GROUPS = ((0, 4), (4, 8), (8, 12))
RM_HALF_FROM = 12

_CACHE = {}

# exec time of the last device run (ns), populated when tracing is on
last_exec_time_ns = None


def _build():
    import concourse.bass as bass
    import concourse.tile as tile
    from concourse import bacc, mybir

    dt = mybir.dt
    DRI = mybir.MatmulPerfMode.DoubleRowSwInterleave
    nc = bacc.Bacc(
        "TRN2", target_bir_lowering=False, debug=False, num_devices=NCORES
    )

    # ---- DRAM I/O (all partition-major so every DMA is [128, N] flat) ----
    zq_d = nc.dram_tensor("zq", [128, 8, 256], dt.float8e4, kind="ExternalInput").ap()
    qt_d = [
        nc.dram_tensor(
            f"qt{h}", [128, JSUB, KPAIR, 256], dt.float8e4, kind="ExternalInput"
        ).ap()
        for h in range(QCHUNK)
    ]
    # head = cTb0 | cTb1 | cTa0a in one transfer (lands in one burst)
    head_d = nc.dram_tensor(
        "head", [128, 2 * KSUB * CW + KSUB * 2 * CW], dt.float8e4,
        kind="ExternalInput",
    ).ap()
    cTa0b_d = nc.dram_tensor(
        "cTa0b", [128, KSUB, 2 * CW], dt.float8e4, kind="ExternalInput"
    ).ap()
    cTa1_d = nc.dram_tensor(
        "cTa1", [128, KSUB, 4 * CW], dt.float8e4, kind="ExternalInput"
    ).ap()
    cTa2_d = nc.dram_tensor(
        "cTa2", [128, KSUB, 4 * CW], dt.float8e4, kind="ExternalInput"
    ).ap()

    # merged output, three contiguous DRAM tensors:
    # outA = s2 chunks 0-5, outB = chunks 6-11, outC = chunks 12-13|rm|fin
    outA_d = nc.dram_tensor(
        "outA", [128, OUTA_END], dt.float8e4, kind="ExternalOutput"
    ).ap()
    outB_d = nc.dram_tensor(
        "outB", [128, OUTB_END - OUTA_END], dt.float8e4, kind="ExternalOutput"
    ).ap()
    outC_d = nc.dram_tensor(
        "outC", [128, OUT_BYTES - OUTB_END], dt.float8e4, kind="ExternalOutput"
    ).ap()

    with tile.TileContext(nc) as tc:
        with (
            tc.tile_pool(name="const", bufs=1) as cpool,
            tc.tile_pool(name="ps1", bufs=2, space="PSUM") as ps1,
            tc.tile_pool(name="psum1s", bufs=1, space="PSUM") as ps1s,
            tc.tile_pool(name="ps2", bufs=3, space="PSUM") as ps2,
        ):
            # ---- resident SBUF tensors ----
            zq_sb = cpool.tile([128, 8, 256], dt.float8e4)
            head_sb = cpool.tile([128, 4 * KSUB * CW], dt.float8e4, tag="head")
            cTb_sb = [
                head_sb[:, i * KSUB * CW : (i + 1) * KSUB * CW].rearrange(
                    "p (k w) -> p k w", k=KSUB, w=CW
                )
                for i in range(2)
            ]
            cTa0a_sb = head_sb[:, 2 * KSUB * CW :].rearrange(
                "p (k w) -> p k w", k=KSUB, w=2 * CW
            )
            cTa0b_sb = cpool.tile([128, KSUB, 2 * CW], dt.float8e4, tag="cTa0b")
            cTa1_sb = cpool.tile([128, KSUB, 4 * CW], dt.float8e4, tag="cTa1")
            cTa2_sb = cpool.tile([128, KSUB, 4 * CW], dt.float8e4, tag="cTa2")
            qt_sb = [
                cpool.tile(
                    [128, JSUB, KPAIR, 256], dt.float8e4, name=f"qt{h}", tag=f"qt{h}"
                )
                for h in range(QCHUNK)
            ]

            # ALL input DMAs on the sync HWDGE ring: it has strict priority
            # over the scalar ring, so one FIFO in exact consumption order
            # gives full bandwidth to the critical stream and exact arrival
            # order.  Exports ride gpsimd (SWDGE) except the final outC.
            nc.sync.dma_start(zq_sb[:], zq_d[:])
            nc.sync.dma_start(head_sb[:], head_d[:])
            nc.sync.dma_start(qt_sb[0][:], qt_d[0][:])
            nc.sync.dma_start(cTa0b_sb[:], cTa0b_d[:])
            nc.sync.dma_start(qt_sb[1][:], qt_d[1][:])
            nc.sync.dma_start(qt_sb[2][:], qt_d[2][:])
            nc.sync.dma_start(qt_sb[3][:], qt_d[3][:])
            nc.sync.dma_start(cTa1_sb[:], cTa1_d[:])
            nc.sync.dma_start(cTa2_sb[:], cTa2_d[:])

            # interleave/column-reversal of all-ones is all-ones
            ones_sb = cpool.tile([128, 256], dt.float8e4)
            nc.vector.memset(ones_sb[:], 1.0)

            # merged export tile + typed views
            out_sb = cpool.tile([128, OUT_BYTES], dt.float8e4)
            s2_v = out_sb[:, 0:S2_BYTES].rearrange(
                "p (c b w) -> p c b w", c=CCH, b=2, w=CW
            )
            rm_v = out_sb[:, RM_OFF:FIN_OFF].bitcast(dt.float32)   # [128, 64]
            fin_v = out_sb[:, FIN_OFF:OUT_BYTES].bitcast(dt.float32)  # [128, 256]

            exp_all = cpool.tile([128, NBT, 4, B], dt.float8e4)
            exp_tiles = [exp_all[:, t] for t in range(NBT)]

            fin_sb = None  # ACT writes fin via fin_v

            # ---- emission helpers ----
            def emit_chunk(ch):
                """part-2 chunk: s2[:, ch] = Z_q @ c_chunk.T, cast to fp8."""
                if ch < 2:
                    cmov, w = cTb_sb[ch], 0
                elif ch < 4:
                    cmov, w = cTa0a_sb, ch - 2
                elif ch < 6:
                    cmov, w = cTa0b_sb, ch - 4
                elif ch < 10:
                    cmov, w = cTa1_sb, ch - 6
                else:
                    cmov, w = cTa2_sb, ch - 10
                for bt in range(2):
                    s2_ps = ps2.tile([128, CW], dt.float32, tag="s2")
                    for kp in range(KPAIR):
                        nc.tensor.matmul(
                            s2_ps[:],
                            zq_sb[:, 4 + 2 * kp + bt, :],
                            cmov[:, 2 * kp : 2 * kp + 2, w * CW : (w + 1) * CW],
                            start=(kp == 0),
                            stop=(kp == KPAIR - 1),
                            perf_mode=DRI,
                        )
                    if ch in DVE_CAST_CH:
                        nc.vector.tensor_copy(s2_v[:, ch, bt, :], s2_ps[:])
                    else:
                        nc.scalar.copy(s2_v[:, ch, bt, :], s2_ps[:])

            rm_after = {b - 1: (a, b) for a, b in RM_GROUPS}

            def emit_tile(t, p1s_ps):
                """part-1 tile: 512 queue rows -> exp fp8; lagged ones-sum;
                rm reduce at group boundaries (per half-tile near the end)."""
                s1_ps = ps1.tile([128, 4, B], dt.float32, tag="s1")
                for q in range(4):
                    jt = t * 4 + q
                    h, jl = divmod(jt, JSUB)
                    for kp in range(KPAIR):
                        nc.tensor.matmul(
                            s1_ps[:, q, :],
                            qt_sb[h][:, jl, kp, :],
                            zq_sb[:, 2 * kp : 2 * kp + 2, :],
                            start=(kp == 0),
                            stop=(kp == KPAIR - 1),
                            perf_mode=DRI,
                        )
                nc.scalar.activation(
                    exp_tiles[t][:],
                    s1_ps[:],
                    mybir.ActivationFunctionType.Exp,
                    scale=1.0 / (DOT_SCALE * INFO_TEMP),
                )
                if t > 1:
                    # two tiles behind: ACT's exp has a full tile of slack
                    for g in range(2):
                        nc.tensor.matmul(
                            p1s_ps[:],
                            ones_sb[:],
                            exp_tiles[t - 2][:, 2 * g : 2 * g + 2, :],
                            start=(t == 2 and g == 0),
                            stop=False,
                            perf_mode=DRI,
                        )
                if t >= RM_HALF_FROM:
                    for hh in range(2):
                        nc.vector.tensor_reduce(
                            rm_v[:, t * 4 + 2 * hh : t * 4 + 2 * hh + 2],
                            exp_all[:, t, 2 * hh : 2 * hh + 2],
                            axis=mybir.AxisListType.X,
                            op=mybir.AluOpType.max,
                        )
                elif t in rm_after:
                    a, b = rm_after[t]
                    nc.vector.tensor_reduce(
                        rm_v[:, a * 4 : b * 4],
                        exp_all[:, a:b],
                        axis=mybir.AxisListType.X,
                        op=mybir.AluOpType.max,
                    )

            # ---- emission: warmup, head chunks, part-1 backbone, tail ----
            p1s_ps = ps1s.tile([128, B], dt.float32)   # sum-of-exp accum

            # HAM warmup: ~3.4us of dummy matmuls on zq (the first input to
            # land) so the real work runs at 2.4 GHz from the start.  The
            # results are never read; the psum pool recycles the banks.
            for i in range(16):
                w_ps = ps2.tile([128, CW], dt.float32, tag="s2")
                nc.tensor.matmul(
                    w_ps[:, 0:B],
                    zq_sb[:, 4, :],
                    zq_sb[:, 0:2, :],
                    start=True,
                    stop=True,
                    perf_mode=DRI,
                )

            for ch in range(4):
                emit_chunk(ch)
            for t in range(4):
                emit_tile(t, p1s_ps)
            emit_chunk(4)
            emit_chunk(5)
            for t in range(4, NBT):
                emit_tile(t, p1s_ps)
            # outA: chunks 0-5 (overlaps the remaining compute)
            nc.gpsimd.dma_start(outA_d[:], out_sb[:, 0:OUTA_END])
            # chunks 6-7 keep the PE busy while ACT finishes exp t14/t15
            # (the lagged ones-flush below waits on those exps)
            emit_chunk(6)
            emit_chunk(7)
            # flush the lagged ones-sum (tiles 14, 15)
            for t in (NBT - 2, NBT - 1):
                for g in range(2):
                    nc.tensor.matmul(
                        p1s_ps[:],
                        ones_sb[:],
                        exp_tiles[t][:, 2 * g : 2 * g + 2, :],
                        start=False,
                        stop=(t == NBT - 1 and g == 1),
                        perf_mode=DRI,
                    )
            nc.scalar.copy(fin_v[:], p1s_ps[:])
            emit_chunk(8)
            emit_chunk(9)
            nc.gpsimd.dma_start(outB_d[:], out_sb[:, OUTA_END:OUTB_END])
            for ch in range(10, CCH):
                emit_chunk(ch)
            # final export: chunks 10-13 + rm + fin (4864 B lines)
            nc.sync.dma_start(outC_d[:], out_sb[:, OUTB_END:OUT_BYTES])

    nc.compile()
    return nc


def _get_nc():
    if "nc" not in _CACHE:
        _CACHE["nc"] = _build()
    return _CACHE["nc"]


def _to_fp8(x):
    return (x * FP8_SCALE).astype(ml_dtypes.float8_e4m3fn)


def _interleave(A, B):
    """SwInterleave weight layout: mem[p, 2*jj+i] = layer_i[p, 127-jj].
    A, B: [..., 128, 128] (partition, column)."""
    return np.stack([A[..., ::-1], B[..., ::-1]], axis=-1).reshape(
        *A.shape[:-1], 256
    )


def _prep_inputs(Z_q, queue, centroids):
    """Host-side shard prep: x16 scale + e4m3 quantization + transpose to
    [C, rows], then partition-major chunk layouts so each DMA is a flat
    [128, N].  Stationary operands are pre-interleaved for SwInterleave."""
    zqT8 = _to_fp8(Z_q).T                            # [512, 256]
    zqT = zqT8.reshape(KSUB, 128, B).transpose(1, 0, 2)  # [128, KSUB, B]
    # part-2 stationary: [128, kp*2+bt, 256] interleaved
    zz = zqT8.reshape(KPAIR, 2, 128, 2, 128)         # [kp, i, p, bt, col]
    zqTi = (
        _interleave(zz[:, 0], zz[:, 1])
        .transpose(1, 0, 2, 3)
        .reshape(128, KSUB, 256)
    )
    zq = np.ascontiguousarray(np.concatenate([zqT, zqTi], axis=1))  # [128, 8, 256]

    qT = np.ascontiguousarray(_to_fp8(queue).T)      # [512, 65536]
    cT = np.ascontiguousarray(_to_fp8(centroids).T)  # [512, 50000]

    in_maps = []
    for i in range(NCORES):
        q_sh = qT[:, i * QSH : (i + 1) * QSH]        # [512, 8192]
        # [kp, i, p, h, jl, col]
        qq = q_sh.reshape(KPAIR, 2, 128, QCHUNK, JSUB, 128)
        q_sh = np.ascontiguousarray(
            _interleave(qq[:, 0], qq[:, 1]).transpose(2, 1, 3, 0, 4)
        )  # [QCHUNK, 128, JSUB, KPAIR, 256]
        qt_chunks = {f"qt{h}": q_sh[h] for h in range(QCHUNK)}
        c_sh = np.zeros((C, CSH_PAD), ml_dtypes.float8_e4m3fn)
        c_sh[:, :CSH] = cT[:, i * CSH : (i + 1) * CSH]
        # cTb0/cTb1 = first 2 matmul chunks (small, land first); cTa = rest
        def cpack(lo, hi):
            return np.ascontiguousarray(
                c_sh[:, lo * CW : hi * CW]
                .reshape(KSUB, 128, (hi - lo) * CW)
                .transpose(1, 0, 2)
            )

        head = np.ascontiguousarray(
            np.concatenate(
                [cpack(0, 1).reshape(128, -1), cpack(1, 2).reshape(128, -1),
                 cpack(2, 4).reshape(128, -1)],
                axis=1,
            )
        )  # [128, 7168]: cTb0 | cTb1 | cTa0a per partition
        in_maps.append({"zq": zq, **qt_chunks, "head": head,
                        "cTa0b": cpack(4, 6),
                        "cTa1": cpack(6, 10), "cTa2": cpack(10, 14)})
    return in_maps


def kernel(Z_q, Z_k, queue, centroids, kmeans_temp, neg_raw):
    global last_exec_time_ns
    from concourse.bass_utils import run_bass_kernel_spmd

    nc = _get_nc()
    in_maps = _prep_inputs(Z_q, queue, centroids)

    trace = bool(int(os.environ.get("MOCO_BASS_TRACE", "0")))
    out = run_bass_kernel_spmd(nc, in_maps, core_ids=list(range(NCORES)), trace=trace)
    last_exec_time_ns = out.exec_time_ns
    res = out.results

    # decode the merged export per core
    def regions(r):
        raw = np.concatenate([r["outA"], r["outB"], r["outC"]], axis=1)
        s2 = raw[:, :S2_BYTES].astype(np.float32).reshape(128, CCH, 2, CW)
        rm = np.ascontiguousarray(raw[:, RM_OFF:FIN_OFF]).view(np.float32)
        fin = np.ascontiguousarray(raw[:, FIN_OFF:]).view(np.float32)
        return s2, rm, fin

    decoded = [regions(r) for r in res]

    # ---- host combine (tiny) ----
    lp = (Z_q.astype(np.float64) * Z_k.astype(np.float64)).sum(axis=1)  # l_pos
    lp_t = lp / INFO_TEMP

    # part-1 loss: logsumexp over [l_pos | l_neg]/T per batch row.
    # Device partials are unshifted sums of e^{s/T} (|s/T| <= ~4).
    S = np.zeros(B, np.float64)
    for _, _, fin in decoded:
        S += fin[0].astype(np.float64)
    S += np.exp(lp_t)
    lse1 = np.log(S)
    loss1 = np.mean(lse1 - lp_t)

    # accuracy: exact despite fp8 scores.  The device reduces the fp8 exp
    # tiles over the batch axis (rm = max_b exp(s/T), fp32-exact); every
    # row with margin < MARGIN is re-checked on the host in full precision.
    rm_full = np.empty(QUEUE, np.float64)
    for i, (_, rm, _) in enumerate(decoded):
        # rm[p, jt] -> queue row j = jt*128 + p
        rm_full[i * QSH : (i + 1) * QSH] = (
            np.log(rm.astype(np.float64).T.reshape(-1)) * INFO_TEMP
        )

    # s0 computed exactly on the host (33 MFLOP) -- only rm comes from
    # the device, so the margin test has one noisy side instead of two
    s0_full = queue.astype(np.float64) @ Z_q[0].astype(np.float64)
    cand = (rm_full - s0_full) < MARGIN
    cols = np.nonzero(cand)[0]
    sub = Z_q.astype(np.float64) @ queue[cols].astype(np.float64).T  # [B, ncand]
    count = float((sub[0] >= sub.max(axis=0)).sum())
    count += float(lp[0] >= lp.max())
    accuracy = count / (1 + QUEUE)

    # part-2: global argmax over centroids (== argmin of ||c||^2 - 2 s).
    # s2 arrives in fp8; the argmax (and the positive logit) is resolved
    # exactly by re-checking every near-max column in full precision.
    s2_full = np.empty((B, NCL), np.float32)
    for i, (s2, _, _) in enumerate(decoded):
        sh = s2.transpose(2, 0, 1, 3).reshape(B, CSH_PAD)
        s2_full[:, i * CSH : (i + 1) * CSH] = sh[:, :CSH]
    s2_full /= DOT_SCALE

    kt = kmeans_temp.astype(np.float64)
    Zq64 = Z_q.astype(np.float64)
    ce64 = centroids.astype(np.float64)
    mx = s2_full.max(axis=1)
    I = np.empty(B, np.int64)
    pl_pos = np.empty(B)
    for b in range(B):
        cnd = np.nonzero(s2_full[b] >= mx[b] - MARGIN2)[0]
        ex = ce64[cnd] @ Zq64[b]
        k = int(np.argmax(ex))
        I[b] = cnd[k]
        pl_pos[b] = ex[k] / kt[cnd[k]]

    neg_idx = neg_raw + (neg_raw >= I[:, None]).astype(neg_raw.dtype)
    pl_neg = (
        np.take_along_axis(s2_full, neg_idx, axis=1).astype(np.float64)
        / kt[neg_idx]
    )
    plogits = np.concatenate([pl_pos[:, None], pl_neg], axis=1)
    m = plogits.max(axis=1)
    plse = np.log(np.exp(plogits - m[:, None]).sum(axis=1)) + m
    ploss = np.mean(plse - pl_pos)

    loss = loss1 + PROTO_FACTOR * ploss
    return np.float32(loss), np.float32(accuracy)


# revision 47
# speedup vs baseline: 1.0315x; 1.0121x over previous
"""MoCo loss (InfoNCE over a 65536-entry queue + proto-NCE over 50000
k-means centroids) on 8 Trainium2 NeuronCores.

fp8e4m3 operands with DoubleRowSwInterleave matmuls (2 contraction
subtiles per instruction; the stationary operand is pre-interleaved on
the host -- A/B k-layer pairs per column, columns reversed -- so the
weight load reads contiguously).  Tables are scaled by 16 per operand
(dots come out x256) and quantized to e4m3 on the host.

Per core (tables sharded by row, Z_q replicated):

  part 2 (centroid shard, zq stationary / centroids moving):
    - s2 = Z_q @ c_shard.T  (PE, fp32 acc), cast fp8 (DVE/ACT), export
      (argmax + exclusion gather on the host)
  part 1 (queue shard, queue stationary / zq moving):
    - s1 = q_shard @ Z_q.T                 (PE)
    - exp(s1/T) in fp8  (no shift; values in [e^-4, e^4])   (ACT)
    - per-queue-row max of the exp tiles -> rm export       (DVE)
    - ones DoubleRow matmul partition sum -> per-batch partial
      sum-of-exp                                            (PE)

The kernel streams ~7.3 MiB of table reads per core on both HWDGE
rings (sync + scalar); part-1 tiles and part-2 chunks are emitted
interleaved in DMA-arrival order so the PE never starves; part-2's
last chunks are the final PE work so the end-of-kernel chain is just
cast -> one merged export (s2 tail + rm + fin share one DRAM tensor
with fat 6.5 KiB lines).

The host combines per-core partials (logsumexp merge, global argmax,
exclusion gather + 513-wide softmax).  Both outputs stay exact despite
fp8 noise: the host computes s0 = queue @ Z_q[0] itself and re-checks
every queue row within MARGIN of the device rowmax -- and every
centroid column within MARGIN2 of its row max -- with full-precision
dot products (fp8 dot error is < 0.01).
"""

import os
import numpy as np
import ml_dtypes

B, C = 256, 512
QUEUE, NCL, NNEG = 65536, 50000, 512
INFO_TEMP = 0.07
PROTO_FACTOR = 0.5
NCORES = 8
QSH = QUEUE // NCORES          # 8192 queue rows per core
CSH = NCL // NCORES            # 6250 centroid rows per core
CSH_PAD = 6272                 # 14 * 448
CCH = 14                       # s2 matmul chunks
CW = CSH_PAD // CCH            # 448
KSUB = C // 128                # 4 contraction subtiles
KPAIR = KSUB // 2              # 2 DoubleRow pairs
NBT = 16                       # part-1 big tiles (512 queue rows each)
QCHUNK = 4                     # qT DMA chunks (8 KiB per-partition lines)
JW = QSH // QCHUNK             # 2048
JSUB = JW // 128               # 16 queue subtiles per chunk
FP8_SCALE = 16.0               # per-operand scale; dots come out x256
DOT_SCALE = FP8_SCALE * FP8_SCALE
MARGIN = 0.05                  # host re-check threshold (unscaled units)
MARGIN2 = 0.045                # part-2 argmax re-check threshold

# merged export regions (bytes per partition inside the SBUF out tile).
# DRAM side is THREE separate contiguous tensors (outA/outB/outC) --
# strided DRAM writes run ~4x slower than contiguous ones.
S2_BYTES = CCH * 2 * CW        # 12544
RM_OFF = S2_BYTES              # [128, NBT*4] fp32 = 256 B
FIN_OFF = RM_OFF + NBT * 4 * 4  # [128, 256] fp32 = 1024 B
OUT_BYTES = FIN_OFF + B * 4    # 13824
OUTA_END = 6 * 2 * CW          # chunks 0-5   (5376 B lines)
OUTB_END = 10 * 2 * CW         # chunks 6-9   (3584 B lines)
# outC = chunks 10-13 + rm + fin (4864 B lines, the final export)

# s2 cast engines: head chunks + late-tail evens on DVE, the rest on ACT
# (ACT covers the early tail while DVE finishes the rm reduces)
DVE_CAST_CH = frozenset((0, 1, 2, 3, 4, 5, 10, 12))
# rm reduce groups: big groups while exps stream; the last 4 tiles reduce
# per half-tile so the post-exp tail chain is one 512-elem reduce
RM_GROUPS = ((0, 4), (4, 8), (8, 12))
RM_HALF_FROM = 12

_CACHE = {}

# exec time of the last device run (ns), populated when tracing is on
last_exec_time_ns = None


def _build():
    import concourse.bass as bass
    import concourse.tile as tile
    from concourse import bacc, mybir

    dt = mybir.dt
    DRI = mybir.MatmulPerfMode.DoubleRowSwInterleave
    nc = bacc.Bacc(
        "TRN2", target_bir_lowering=False, debug=False, num_devices=NCORES
    )

    # ---- DRAM I/O (all partition-major so every DMA is [128, N] flat) ----
    zq_d = nc.dram_tensor("zq", [128, 8, 256], dt.float8e4, kind="ExternalInput").ap()
    qt_d = [
        nc.dram_tensor(
            f"qt{h}", [128, JSUB, KPAIR, 256], dt.float8e4, kind="ExternalInput"
        ).ap()
        for h in range(QCHUNK)
    ]
    # head = cTb0 | cTb1 | cTa0a in one transfer (lands in one burst)
    head_d = nc.dram_tensor(
        "head", [128, 2 * KSUB * CW + KSUB * 2 * CW], dt.float8e4,
        kind="ExternalInput",
    ).ap()
    cTa0b_d = nc.dram_tensor(
        "cTa0b", [128, KSUB, 2 * CW], dt.float8e4, kind="ExternalInput"
    ).ap()
    cTa1_d = nc.dram_tensor(
        "cTa1", [128, KSUB, 4 * CW], dt.float8e4, kind="ExternalInput"
    ).ap()
    cTa2_d = nc.dram_tensor(
        "cTa2", [128, KSUB, 4 * CW], dt.float8e4, kind="ExternalInput"
    ).ap()

    # merged output, three contiguous DRAM tensors:
    # outA = s2 chunks 0-5, outB = chunks 6-11, outC = chunks 12-13|rm|fin
    outA_d = nc.dram_tensor(
        "outA", [128, OUTA_END], dt.float8e4, kind="ExternalOutput"
    ).ap()
    outB_d = nc.dram_tensor(
        "outB", [128, OUTB_END - OUTA_END], dt.float8e4, kind="ExternalOutput"
    ).ap()
    outC_d = nc.dram_tensor(
        "outC", [128, S2_BYTES - OUTB_END], dt.float8e4, kind="ExternalOutput"
    ).ap()
    outD_d = nc.dram_tensor(
        "outD", [128, OUT_BYTES - S2_BYTES], dt.float8e4, kind="ExternalOutput"
    ).ap()

    with tile.TileContext(nc) as tc:
        with (
            tc.tile_pool(name="const", bufs=1) as cpool,
            tc.tile_pool(name="ps1", bufs=2, space="PSUM") as ps1,
            tc.tile_pool(name="psum1s", bufs=1, space="PSUM") as ps1s,
            tc.tile_pool(name="ps2", bufs=3, space="PSUM") as ps2,
        ):
            # ---- resident SBUF tensors ----
            zq_sb = cpool.tile([128, 8, 256], dt.float8e4)
            head_sb = cpool.tile([128, 4 * KSUB * CW], dt.float8e4, tag="head")
            cTb_sb = [
                head_sb[:, i * KSUB * CW : (i + 1) * KSUB * CW].rearrange(
                    "p (k w) -> p k w", k=KSUB, w=CW
                )
                for i in range(2)
            ]
            cTa0a_sb = head_sb[:, 2 * KSUB * CW :].rearrange(
                "p (k w) -> p k w", k=KSUB, w=2 * CW
            )
            cTa0b_sb = cpool.tile([128, KSUB, 2 * CW], dt.float8e4, tag="cTa0b")
            cTa1_sb = cpool.tile([128, KSUB, 4 * CW], dt.float8e4, tag="cTa1")
            cTa2_sb = cpool.tile([128, KSUB, 4 * CW], dt.float8e4, tag="cTa2")
            qt_sb = [
                cpool.tile(
                    [128, JSUB, KPAIR, 256], dt.float8e4, name=f"qt{h}", tag=f"qt{h}"
                )
                for h in range(QCHUNK)
            ]

            # ALL input DMAs on the sync HWDGE ring: it has strict priority
            # over the scalar ring, so one FIFO in exact consumption order
            # gives full bandwidth to the critical stream and exact arrival
            # order.  Exports ride gpsimd (SWDGE) except the final outC.
            nc.sync.dma_start(zq_sb[:], zq_d[:])
            nc.sync.dma_start(head_sb[:], head_d[:])
            nc.sync.dma_start(qt_sb[0][:], qt_d[0][:])
            nc.sync.dma_start(cTa0b_sb[:], cTa0b_d[:])
            nc.sync.dma_start(qt_sb[1][:], qt_d[1][:])
            nc.sync.dma_start(qt_sb[2][:], qt_d[2][:])
            nc.sync.dma_start(qt_sb[3][:], qt_d[3][:])
            nc.sync.dma_start(cTa1_sb[:], cTa1_d[:])
            nc.sync.dma_start(cTa2_sb[:], cTa2_d[:])

            # interleave/column-reversal of all-ones is all-ones
            ones_sb = cpool.tile([128, 256], dt.float8e4)
            nc.vector.memset(ones_sb[:], 1.0)

            # merged export tile + typed views
            out_sb = cpool.tile([128, OUT_BYTES], dt.float8e4)
            s2_v = out_sb[:, 0:S2_BYTES].rearrange(
                "p (c b w) -> p c b w", c=CCH, b=2, w=CW
            )
            rm_v = out_sb[:, RM_OFF:FIN_OFF].bitcast(dt.float32)   # [128, 64]
            fin_v = out_sb[:, FIN_OFF:OUT_BYTES].bitcast(dt.float32)  # [128, 256]

            exp_all = cpool.tile([128, NBT, 4, B], dt.float8e4)
            exp_tiles = [exp_all[:, t] for t in range(NBT)]

            fin_sb = None  # ACT writes fin via fin_v

            # ---- emission helpers ----
            def emit_chunk(ch):
                """part-2 chunk: s2[:, ch] = Z_q @ c_chunk.T, cast to fp8."""
                if ch < 2:
                    cmov, w = cTb_sb[ch], 0
                elif ch < 4:
                    cmov, w = cTa0a_sb, ch - 2
                elif ch < 6:
                    cmov, w = cTa0b_sb, ch - 4
                elif ch < 10:
                    cmov, w = cTa1_sb, ch - 6
                else:
                    cmov, w = cTa2_sb, ch - 10
                for bt in range(2):
                    s2_ps = ps2.tile([128, CW], dt.float32, tag="s2")
                    for kp in range(KPAIR):
                        nc.tensor.matmul(
                            s2_ps[:],
                            zq_sb[:, 4 + 2 * kp + bt, :],
                            cmov[:, 2 * kp : 2 * kp + 2, w * CW : (w + 1) * CW],
                            start=(kp == 0),
                            stop=(kp == KPAIR - 1),
                            perf_mode=DRI,
                        )
                    if ch == CCH - 1:
                        # the very last chunk: split its two casts across
                        # both engines so the final export issues sooner
                        eng = nc.vector.tensor_copy if bt == 0 else nc.scalar.copy
                        eng(s2_v[:, ch, bt, :], s2_ps[:])
                    elif ch in DVE_CAST_CH:
                        nc.vector.tensor_copy(s2_v[:, ch, bt, :], s2_ps[:])
                    else:
                        nc.scalar.copy(s2_v[:, ch, bt, :], s2_ps[:])

            rm_after = {b - 1: (a, b) for a, b in RM_GROUPS}

            def emit_tile(t, p1s_ps):
                """part-1 tile: 512 queue rows -> exp fp8; lagged ones-sum;
                rm reduce at group boundaries (per half-tile near the end)."""
                s1_ps = ps1.tile([128, 4, B], dt.float32, tag="s1")
                for q in range(4):
                    jt = t * 4 + q
                    h, jl = divmod(jt, JSUB)
                    for kp in range(KPAIR):
                        nc.tensor.matmul(
                            s1_ps[:, q, :],
                            qt_sb[h][:, jl, kp, :],
                            zq_sb[:, 2 * kp : 2 * kp + 2, :],
                            start=(kp == 0),
                            stop=(kp == KPAIR - 1),
                            perf_mode=DRI,
                        )
                nc.scalar.activation(
                    exp_tiles[t][:],
                    s1_ps[:],
                    mybir.ActivationFunctionType.Exp,
                    scale=1.0 / (DOT_SCALE * INFO_TEMP),
                )
                if t > 1:
                    # two tiles behind: ACT's exp has a full tile of slack
                    for g in range(2):
                        nc.tensor.matmul(
                            p1s_ps[:],
                            ones_sb[:],
                            exp_tiles[t - 2][:, 2 * g : 2 * g + 2, :],
                            start=(t == 2 and g == 0),
                            stop=False,
                            perf_mode=DRI,
                        )
                if t >= RM_HALF_FROM:
                    for hh in range(2):
                        nc.vector.tensor_reduce(
                            rm_v[:, t * 4 + 2 * hh : t * 4 + 2 * hh + 2],
                            exp_all[:, t, 2 * hh : 2 * hh + 2],
                            axis=mybir.AxisListType.X,
                            op=mybir.AluOpType.max,
                        )
                elif t in rm_after:
                    a, b = rm_after[t]
                    nc.vector.tensor_reduce(
                        rm_v[:, a * 4 : b * 4],
                        exp_all[:, a:b],
                        axis=mybir.AxisListType.X,
                        op=mybir.AluOpType.max,
                    )

            # ---- emission: warmup, head chunks, part-1 backbone, tail ----
            p1s_ps = ps1s.tile([128, B], dt.float32)   # sum-of-exp accum

            # HAM warmup: ~3.4us of dummy matmuls on zq (the first input to
            # land) so the real work runs at 2.4 GHz from the start.  The
            # results are never read; the psum pool recycles the banks.
            for i in range(16):
                w_ps = ps2.tile([128, CW], dt.float32, tag="s2")
                nc.tensor.matmul(
                    w_ps[:, 0:B],
                    zq_sb[:, 4, :],
                    zq_sb[:, 0:2, :],
                    start=True,
                    stop=True,
                    perf_mode=DRI,
                )

            for ch in range(4):
                emit_chunk(ch)
            for t in range(4):
                emit_tile(t, p1s_ps)
            emit_chunk(4)
            emit_chunk(5)
            for t in range(4, NBT):
                emit_tile(t, p1s_ps)
            # outA: chunks 0-5 (overlaps the remaining compute)
            nc.gpsimd.dma_start(outA_d[:], out_sb[:, 0:OUTA_END])
            # chunks 6-7 keep the PE busy while ACT finishes exp t14/t15
            # (the lagged ones-flush below waits on those exps)
            emit_chunk(6)
            emit_chunk(7)
            # flush the lagged ones-sum (tiles 14, 15)
            for t in (NBT - 2, NBT - 1):
                for g in range(2):
                    nc.tensor.matmul(
                        p1s_ps[:],
                        ones_sb[:],
                        exp_tiles[t][:, 2 * g : 2 * g + 2, :],
                        start=False,
                        stop=(t == NBT - 1 and g == 1),
                        perf_mode=DRI,
                    )
            nc.scalar.copy(fin_v[:], p1s_ps[:])
            # rm + fin are ready well before the last chunks: export now
            nc.sync.dma_start(outD_d[:], out_sb[:, S2_BYTES:OUT_BYTES])
            emit_chunk(8)
            emit_chunk(9)
            nc.gpsimd.dma_start(outB_d[:], out_sb[:, OUTA_END:OUTB_END])
            for ch in range(10, CCH):
                emit_chunk(ch)
            # final export: chunks 10-13 (3584 B lines)
            nc.sync.dma_start(outC_d[:], out_sb[:, OUTB_END:S2_BYTES])

    nc.compile()
    return nc


def _get_nc():
    if "nc" not in _CACHE:
        _CACHE["nc"] = _build()
    return _CACHE["nc"]


def _to_fp8(x):
    return (x * FP8_SCALE).astype(ml_dtypes.float8_e4m3fn)


def _interleave(A, B):
    """SwInterleave weight layout: mem[p, 2*jj+i] = layer_i[p, 127-jj].
    A, B: [..., 128, 128] (partition, column)."""
    return np.stack([A[..., ::-1], B[..., ::-1]], axis=-1).reshape(
        *A.shape[:-1], 256
    )


def _prep_inputs(Z_q, queue, centroids):
    """Host-side shard prep: x16 scale + e4m3 quantization + transpose to
    [C, rows], then partition-major chunk layouts so each DMA is a flat
    [128, N].  Stationary operands are pre-interleaved for SwInterleave."""
    zqT8 = _to_fp8(Z_q).T                            # [512, 256]
    zqT = zqT8.reshape(KSUB, 128, B).transpose(1, 0, 2)  # [128, KSUB, B]
    # part-2 stationary: [128, kp*2+bt, 256] interleaved
    zz = zqT8.reshape(KPAIR, 2, 128, 2, 128)         # [kp, i, p, bt, col]
    zqTi = (
        _interleave(zz[:, 0], zz[:, 1])
        .transpose(1, 0, 2, 3)
        .reshape(128, KSUB, 256)
    )
    zq = np.ascontiguousarray(np.concatenate([zqT, zqTi], axis=1))  # [128, 8, 256]

    qT = np.ascontiguousarray(_to_fp8(queue).T)      # [512, 65536]
    cT = np.ascontiguousarray(_to_fp8(centroids).T)  # [512, 50000]

    in_maps = []
    for i in range(NCORES):
        q_sh = qT[:, i * QSH : (i + 1) * QSH]        # [512, 8192]
        # [kp, i, p, h, jl, col]
        qq = q_sh.reshape(KPAIR, 2, 128, QCHUNK, JSUB, 128)
        q_sh = np.ascontiguousarray(
            _interleave(qq[:, 0], qq[:, 1]).transpose(2, 1, 3, 0, 4)
        )  # [QCHUNK, 128, JSUB, KPAIR, 256]
        qt_chunks = {f"qt{h}": q_sh[h] for h in range(QCHUNK)}
        c_sh = np.zeros((C, CSH_PAD), ml_dtypes.float8_e4m3fn)
        c_sh[:, :CSH] = cT[:, i * CSH : (i + 1) * CSH]
        # cTb0/cTb1 = first 2 matmul chunks (small, land first); cTa = rest
        def cpack(lo, hi):
            return np.ascontiguousarray(
                c_sh[:, lo * CW : hi * CW]
                .reshape(KSUB, 128, (hi - lo) * CW)
                .transpose(1, 0, 2)
            )

        head = np.ascontiguousarray(
            np.concatenate(
                [cpack(0, 1).reshape(128, -1), cpack(1, 2).reshape(128, -1),
                 cpack(2, 4).reshape(128, -1)],
                axis=1,
            )
        )  # [128, 7168]: cTb0 | cTb1 | cTa0a per partition
        in_maps.append({"zq": zq, **qt_chunks, "head": head,
                        "cTa0b": cpack(4, 6),
                        "cTa1": cpack(6, 10), "cTa2": cpack(10, 14)})
    return in_maps


def kernel(Z_q, Z_k, queue, centroids, kmeans_temp, neg_raw):
    global last_exec_time_ns
    from concourse.bass_utils import run_bass_kernel_spmd

    nc = _get_nc()
    in_maps = _prep_inputs(Z_q, queue, centroids)

    trace = bool(int(os.environ.get("MOCO_BASS_TRACE", "0")))
    out = run_bass_kernel_spmd(nc, in_maps, core_ids=list(range(NCORES)), trace=trace)
    last_exec_time_ns = out.exec_time_ns
    res = out.results

    # decode the merged export per core
    def regions(r):
        raw = np.concatenate([r["outA"], r["outB"], r["outC"], r["outD"]], axis=1)
        s2 = raw[:, :S2_BYTES].astype(np.float32).reshape(128, CCH, 2, CW)
        rm = np.ascontiguousarray(raw[:, RM_OFF:FIN_OFF]).view(np.float32)
        fin = np.ascontiguousarray(raw[:, FIN_OFF:]).view(np.float32)
        return s2, rm, fin

    decoded = [regions(r) for r in res]

    # ---- host combine (tiny) ----
    lp = (Z_q.astype(np.float64) * Z_k.astype(np.float64)).sum(axis=1)  # l_pos
    lp_t = lp / INFO_TEMP

    # part-1 loss: logsumexp over [l_pos | l_neg]/T per batch row.
    # Device partials are unshifted sums of e^{s/T} (|s/T| <= ~4).
    S = np.zeros(B, np.float64)
    for _, _, fin in decoded:
        S += fin[0].astype(np.float64)
    S += np.exp(lp_t)
    lse1 = np.log(S)
    loss1 = np.mean(lse1 - lp_t)

    # accuracy: exact despite fp8 scores.  The device reduces the fp8 exp
    # tiles over the batch axis (rm = max_b exp(s/T), fp32-exact); every
    # row with margin < MARGIN is re-checked on the host in full precision.
    rm_full = np.empty(QUEUE, np.float64)
    for i, (_, rm, _) in enumerate(decoded):
        # rm[p, jt] -> queue row j = jt*128 + p
        rm_full[i * QSH : (i + 1) * QSH] = (
            np.log(rm.astype(np.float64).T.reshape(-1)) * INFO_TEMP
        )

    # s0 computed exactly on the host (33 MFLOP) -- only rm comes from
    # the device, so the margin test has one noisy side instead of two
    s0_full = queue.astype(np.float64) @ Z_q[0].astype(np.float64)
    cand = (rm_full - s0_full) < MARGIN
    cols = np.nonzero(cand)[0]
    sub = Z_q.astype(np.float64) @ queue[cols].astype(np.float64).T  # [B, ncand]
    count = float((sub[0] >= sub.max(axis=0)).sum())
    count += float(lp[0] >= lp.max())
    accuracy = count / (1 + QUEUE)

    # part-2: global argmax over centroids (== argmin of ||c||^2 - 2 s).
    # s2 arrives in fp8; the argmax (and the positive logit) is resolved
    # exactly by re-checking every near-max column in full precision.
    s2_full = np.empty((B, NCL), np.float32)
    for i, (s2, _, _) in enumerate(decoded):
        sh = s2.transpose(2, 0, 1, 3).reshape(B, CSH_PAD)
        s2_full[:, i * CSH : (i + 1) * CSH] = sh[:, :CSH]
    s2_full /= DOT_SCALE

    kt = kmeans_temp.astype(np.float64)
    Zq64 = Z_q.astype(np.float64)
    ce64 = centroids.astype(np.float64)
    mx = s2_full.max(axis=1)
    I = np.empty(B, np.int64)
    pl_pos = np.empty(B)
    for b in range(B):
        cnd = np.nonzero(s2_full[b] >= mx[b] - MARGIN2)[0]
        ex = ce64[cnd] @ Zq64[b]
        k = int(np.argmax(ex))
        I[b] = cnd[k]
        pl_pos[b] = ex[k] / kt[cnd[k]]

    neg_idx = neg_raw + (neg_raw >= I[:, None]).astype(neg_raw.dtype)
    pl_neg = (
        np.take_along_axis(s2_full, neg_idx, axis=1).astype(np.float64)
        / kt[neg_idx]
    )
    plogits = np.concatenate([pl_pos[:, None], pl_neg], axis=1)
    m = plogits.max(axis=1)
    plse = np.log(np.exp(plogits - m[:, None]).sum(axis=1)) + m
    ploss = np.mean(plse - pl_pos)

    loss = loss1 + PROTO_FACTOR * ploss
    return np.float32(loss), np.float32(accuracy)


# revision 48
# speedup vs baseline: 1.0570x; 1.0248x over previous
"""MoCo loss (InfoNCE over a 65536-entry queue + proto-NCE over 50000
k-means centroids) on 8 Trainium2 NeuronCores.

fp8e4m3 operands with DoubleRowSwInterleave matmuls (2 contraction
subtiles per instruction; the stationary operand is pre-interleaved on
the host -- A/B k-layer pairs per column, columns reversed -- so the
weight load reads contiguously).  Tables are scaled by 16 per operand
(dots come out x256) and quantized to e4m3 on the host.

Per core (tables sharded by row, Z_q replicated):

  part 2 (centroid shard, zq stationary / centroids moving):
    - s2 = Z_q @ c_shard.T  (PE, fp32 acc), cast fp8 (DVE/ACT), export
      (argmax + exclusion gather on the host)
  part 1 (queue shard, queue stationary / zq moving):
    - s1 = q_shard @ Z_q.T                 (PE)
    - exp(s1/T) in fp8  (no shift; values in [e^-4, e^4])   (ACT)
    - per-queue-row max of the exp tiles -> rm export       (DVE)
    - ones DoubleRow matmul partition sum -> per-batch partial
      sum-of-exp                                            (PE)

The kernel streams ~7.3 MiB of table reads per core on the sync HWDGE
ring as ONE FIFO in exact consumption order (sync has strict priority
over the scalar ring, so a single queue gives both full bandwidth and
deterministic arrival order).  A ~3.4 us burst of dummy matmuls on zq
warms the PE HAM clock gate before the first real chunk lands.  Part-2
chunks bracket the part-1 backbone (head chunks early, the rest after
the last tile); exports are staged contiguous DRAM tensors -- outA/outB
mid-kernel on gpsimd, rm+fin as soon as they settle, and a short final
chunk export -- so almost nothing trails the compute.

The host combines per-core partials (logsumexp merge, global argmax,
exclusion gather + 513-wide softmax).  Both outputs stay exact despite
fp8 noise: the host computes s0 = queue @ Z_q[0] itself and re-checks
every queue row within MARGIN of the device rowmax -- and every
centroid column within MARGIN2 of its row max -- with full-precision
dot products (fp8 dot error is < 0.01).
"""

import os
import numpy as np
import ml_dtypes

B, C = 256, 512
QUEUE, NCL, NNEG = 65536, 50000, 512
INFO_TEMP = 0.07
PROTO_FACTOR = 0.5
NCORES = 8
QSH = QUEUE // NCORES          # 8192 queue rows per core
CSH = NCL // NCORES            # 6250 centroid rows per core
CSH_PAD = 6272                 # 14 * 448
CCH = 14                       # s2 matmul chunks
CW = CSH_PAD // CCH            # 448
KSUB = C // 128                # 4 contraction subtiles
KPAIR = KSUB // 2              # 2 DoubleRow pairs
NBT = 16                       # part-1 big tiles (512 queue rows each)
QCHUNK = 4                     # qT DMA chunks (8 KiB per-partition lines)
JW = QSH // QCHUNK             # 2048
JSUB = JW // 128               # 16 queue subtiles per chunk
FP8_SCALE = 16.0               # per-operand scale; dots come out x256
DOT_SCALE = FP8_SCALE * FP8_SCALE
MARGIN = 0.05                  # host re-check threshold (unscaled units)
MARGIN2 = 0.045                # part-2 argmax re-check threshold

# merged export regions (bytes per partition inside the SBUF out tile).
# DRAM side is THREE separate contiguous tensors (outA/outB/outC) --
# strided DRAM writes run ~4x slower than contiguous ones.
S2_BYTES = CCH * 2 * CW        # 12544
RM_OFF = S2_BYTES              # [128, NBT*4] fp32 = 256 B
FIN_OFF = RM_OFF + NBT * 4 * 4  # [128, 256] fp32 = 1024 B
OUT_BYTES = FIN_OFF + B * 4    # 13824
OUTA_END = 6 * 2 * CW          # chunks 0-5   (5376 B lines)
OUTB_END = 10 * 2 * CW         # chunks 6-9   (3584 B lines)
# outC = chunks 10-13 + rm + fin (4864 B lines, the final export)

# s2 cast engines: head chunks + late-tail evens on DVE, the rest on ACT
# (ACT covers the early tail while DVE finishes the rm reduces)
DVE_CAST_CH = frozenset((0, 1, 2, 3, 4, 5, 10, 12))
# rm reduce groups: big groups while exps stream; the last 4 tiles reduce
# per half-tile so the post-exp tail chain is one 512-elem reduce
RM_GROUPS = ((0, 4), (4, 8), (8, 12))
RM_HALF_FROM = 12

_CACHE = {}

# exec time of the last device run (ns), populated when tracing is on
last_exec_time_ns = None


def _build():
    import concourse.bass as bass
    import concourse.tile as tile
    from concourse import bacc, mybir

    dt = mybir.dt
    DRI = mybir.MatmulPerfMode.DoubleRowSwInterleave
    nc = bacc.Bacc(
        "TRN2", target_bir_lowering=False, debug=False, num_devices=NCORES
    )

    # ---- DRAM I/O (all partition-major so every DMA is [128, N] flat) ----
    zq_d = nc.dram_tensor("zq", [128, 8, 256], dt.float8e4, kind="ExternalInput").ap()
    qt_d = [
        nc.dram_tensor(
            f"qt{h}", [128, JSUB, KPAIR, 256], dt.float8e4, kind="ExternalInput"
        ).ap()
        for h in range(QCHUNK)
    ]
    # head = cTb0 | cTb1 | cTa0a in one transfer (lands in one burst)
    head_d = nc.dram_tensor(
        "head", [128, 2 * KSUB * CW + KSUB * 2 * CW], dt.float8e4,
        kind="ExternalInput",
    ).ap()
    cTa0b_d = nc.dram_tensor(
        "cTa0b", [128, KSUB, 2 * CW], dt.float8e4, kind="ExternalInput"
    ).ap()
    cTa1_d = nc.dram_tensor(
        "cTa1", [128, KSUB, 4 * CW], dt.float8e4, kind="ExternalInput"
    ).ap()
    cTa2_d = nc.dram_tensor(
        "cTa2", [128, KSUB, 4 * CW], dt.float8e4, kind="ExternalInput"
    ).ap()

    # merged output, three contiguous DRAM tensors:
    # outA = s2 chunks 0-5, outB = chunks 6-11, outC = chunks 12-13|rm|fin
    outA_d = nc.dram_tensor(
        "outA", [128, OUTA_END], dt.float8e4, kind="ExternalOutput"
    ).ap()
    outB_d = nc.dram_tensor(
        "outB", [128, OUTB_END - OUTA_END], dt.float8e4, kind="ExternalOutput"
    ).ap()
    outC_d = nc.dram_tensor(
        "outC", [128, S2_BYTES - OUTB_END], dt.float8e4, kind="ExternalOutput"
    ).ap()
    outD_d = nc.dram_tensor(
        "outD", [128, OUT_BYTES - S2_BYTES], dt.float8e4, kind="ExternalOutput"
    ).ap()

    with tile.TileContext(nc) as tc:
        with (
            tc.tile_pool(name="const", bufs=1) as cpool,
            tc.tile_pool(name="ps1", bufs=2, space="PSUM") as ps1,
            tc.tile_pool(name="psum1s", bufs=1, space="PSUM") as ps1s,
            tc.tile_pool(name="ps2", bufs=3, space="PSUM") as ps2,
        ):
            # ---- resident SBUF tensors ----
            zq_sb = cpool.tile([128, 8, 256], dt.float8e4)
            head_sb = cpool.tile([128, 4 * KSUB * CW], dt.float8e4, tag="head")
            cTb_sb = [
                head_sb[:, i * KSUB * CW : (i + 1) * KSUB * CW].rearrange(
                    "p (k w) -> p k w", k=KSUB, w=CW
                )
                for i in range(2)
            ]
            cTa0a_sb = head_sb[:, 2 * KSUB * CW :].rearrange(
                "p (k w) -> p k w", k=KSUB, w=2 * CW
            )
            cTa0b_sb = cpool.tile([128, KSUB, 2 * CW], dt.float8e4, tag="cTa0b")
            cTa1_sb = cpool.tile([128, KSUB, 4 * CW], dt.float8e4, tag="cTa1")
            cTa2_sb = cpool.tile([128, KSUB, 4 * CW], dt.float8e4, tag="cTa2")
            qt_sb = [
                cpool.tile(
                    [128, JSUB, KPAIR, 256], dt.float8e4, name=f"qt{h}", tag=f"qt{h}"
                )
                for h in range(QCHUNK)
            ]

            # ALL input DMAs on the sync HWDGE ring: it has strict priority
            # over the scalar ring, so one FIFO in exact consumption order
            # gives full bandwidth to the critical stream and exact arrival
            # order.  Exports ride gpsimd (SWDGE) except the final outC.
            nc.sync.dma_start(zq_sb[:], zq_d[:])
            nc.sync.dma_start(head_sb[:], head_d[:])
            nc.sync.dma_start(qt_sb[0][:], qt_d[0][:])
            nc.sync.dma_start(cTa0b_sb[:], cTa0b_d[:])
            nc.sync.dma_start(qt_sb[1][:], qt_d[1][:])
            nc.sync.dma_start(qt_sb[2][:], qt_d[2][:])
            nc.sync.dma_start(qt_sb[3][:], qt_d[3][:])
            nc.sync.dma_start(cTa1_sb[:], cTa1_d[:])
            nc.sync.dma_start(cTa2_sb[:], cTa2_d[:])

            # interleave/column-reversal of all-ones is all-ones
            ones_sb = cpool.tile([128, 256], dt.float8e4)
            nc.vector.memset(ones_sb[:], 1.0)

            # merged export tile + typed views
            out_sb = cpool.tile([128, OUT_BYTES], dt.float8e4)
            s2_v = out_sb[:, 0:S2_BYTES].rearrange(
                "p (c b w) -> p c b w", c=CCH, b=2, w=CW
            )
            rm_v = out_sb[:, RM_OFF:FIN_OFF].bitcast(dt.float32)   # [128, 64]
            fin_v = out_sb[:, FIN_OFF:OUT_BYTES].bitcast(dt.float32)  # [128, 256]

            exp_all = cpool.tile([128, NBT, 4, B], dt.float8e4)
            exp_tiles = [exp_all[:, t] for t in range(NBT)]

            fin_sb = None  # ACT writes fin via fin_v

            # ---- emission helpers ----
            def emit_chunk(ch):
                """part-2 chunk: s2[:, ch] = Z_q @ c_chunk.T, cast to fp8."""
                if ch < 2:
                    cmov, w = cTb_sb[ch], 0
                elif ch < 4:
                    cmov, w = cTa0a_sb, ch - 2
                elif ch < 6:
                    cmov, w = cTa0b_sb, ch - 4
                elif ch < 10:
                    cmov, w = cTa1_sb, ch - 6
                else:
                    cmov, w = cTa2_sb, ch - 10
                for bt in range(2):
                    s2_ps = ps2.tile([128, CW], dt.float32, tag="s2")
                    for kp in range(KPAIR):
                        nc.tensor.matmul(
                            s2_ps[:],
                            zq_sb[:, 4 + 2 * kp + bt, :],
                            cmov[:, 2 * kp : 2 * kp + 2, w * CW : (w + 1) * CW],
                            start=(kp == 0),
                            stop=(kp == KPAIR - 1),
                            perf_mode=DRI,
                        )
                    if ch == CCH - 1:
                        # the very last chunk: split its two casts across
                        # both engines so the final export issues sooner
                        eng = nc.vector.tensor_copy if bt == 0 else nc.scalar.copy
                        eng(s2_v[:, ch, bt, :], s2_ps[:])
                    elif ch in DVE_CAST_CH:
                        nc.vector.tensor_copy(s2_v[:, ch, bt, :], s2_ps[:])
                    else:
                        nc.scalar.copy(s2_v[:, ch, bt, :], s2_ps[:])

            rm_after = {b - 1: (a, b) for a, b in RM_GROUPS}

            def emit_tile(t, p1s_ps):
                """part-1 tile: 512 queue rows -> exp fp8; lagged ones-sum;
                rm reduce at group boundaries (per half-tile near the end)."""
                s1_ps = ps1.tile([128, 4, B], dt.float32, tag="s1")
                for q in range(4):
                    jt = t * 4 + q
                    h, jl = divmod(jt, JSUB)
                    for kp in range(KPAIR):
                        nc.tensor.matmul(
                            s1_ps[:, q, :],
                            qt_sb[h][:, jl, kp, :],
                            zq_sb[:, 2 * kp : 2 * kp + 2, :],
                            start=(kp == 0),
                            stop=(kp == KPAIR - 1),
                            perf_mode=DRI,
                        )
                nc.scalar.activation(
                    exp_tiles[t][:],
                    s1_ps[:],
                    mybir.ActivationFunctionType.Exp,
                    scale=1.0 / (DOT_SCALE * INFO_TEMP),
                )
                if t > 1:
                    # two tiles behind: ACT's exp has a full tile of slack
                    for g in range(2):
                        nc.tensor.matmul(
                            p1s_ps[:],
                            ones_sb[:],
                            exp_tiles[t - 2][:, 2 * g : 2 * g + 2, :],
                            start=(t == 2 and g == 0),
                            stop=False,
                            perf_mode=DRI,
                        )
                if t >= RM_HALF_FROM:
                    for hh in range(2):
                        nc.vector.tensor_reduce(
                            rm_v[:, t * 4 + 2 * hh : t * 4 + 2 * hh + 2],
                            exp_all[:, t, 2 * hh : 2 * hh + 2],
                            axis=mybir.AxisListType.X,
                            op=mybir.AluOpType.max,
                        )
                elif t in rm_after:
                    a, b = rm_after[t]
                    nc.vector.tensor_reduce(
                        rm_v[:, a * 4 : b * 4],
                        exp_all[:, a:b],
                        axis=mybir.AxisListType.X,
                        op=mybir.AluOpType.max,
                    )

            # ---- emission: warmup, head chunks, part-1 backbone, tail ----
            p1s_ps = ps1s.tile([128, B], dt.float32)   # sum-of-exp accum

            # HAM warmup: ~3.4us of dummy matmuls on zq (the first input to
            # land) so the real work runs at 2.4 GHz from the start.  The
            # results are never read; the psum pool recycles the banks.
            for i in range(16):
                w_ps = ps2.tile([128, CW], dt.float32, tag="s2")
                nc.tensor.matmul(
                    w_ps[:, 0:B],
                    zq_sb[:, 4, :],
                    zq_sb[:, 0:2, :],
                    start=True,
                    stop=True,
                    perf_mode=DRI,
                )

            for ch in range(4):
                emit_chunk(ch)
            for t in range(4):
                emit_tile(t, p1s_ps)
            emit_chunk(4)
            emit_chunk(5)
            for t in range(4, NBT):
                emit_tile(t, p1s_ps)
            # outA: chunks 0-5 (overlaps the remaining compute)
            nc.gpsimd.dma_start(outA_d[:], out_sb[:, 0:OUTA_END])
            # chunks 6-7 keep the PE busy while ACT finishes exp t14/t15
            # (the lagged ones-flush below waits on those exps)
            emit_chunk(6)
            emit_chunk(7)
            # flush the lagged ones-sum (tiles 14, 15)
            for t in (NBT - 2, NBT - 1):
                for g in range(2):
                    nc.tensor.matmul(
                        p1s_ps[:],
                        ones_sb[:],
                        exp_tiles[t][:, 2 * g : 2 * g + 2, :],
                        start=False,
                        stop=(t == NBT - 1 and g == 1),
                        perf_mode=DRI,
                    )
            nc.scalar.copy(fin_v[:], p1s_ps[:])
            # rm + fin are ready well before the last chunks: export now
            nc.sync.dma_start(outD_d[:], out_sb[:, S2_BYTES:OUT_BYTES])
            emit_chunk(8)
            emit_chunk(9)
            nc.gpsimd.dma_start(outB_d[:], out_sb[:, OUTA_END:OUTB_END])
            for ch in range(10, CCH):
                emit_chunk(ch)
            # final export: chunks 10-13 (3584 B lines)
            nc.sync.dma_start(outC_d[:], out_sb[:, OUTB_END:S2_BYTES])

    nc.compile()
    return nc


def _get_nc():
    if "nc" not in _CACHE:
        _CACHE["nc"] = _build()
    return _CACHE["nc"]


def _to_fp8(x):
    return (x * FP8_SCALE).astype(ml_dtypes.float8_e4m3fn)


def _interleave(A, B):
    """SwInterleave weight layout: mem[p, 2*jj+i] = layer_i[p, 127-jj].
    A, B: [..., 128, 128] (partition, column)."""
    return np.stack([A[..., ::-1], B[..., ::-1]], axis=-1).reshape(
        *A.shape[:-1], 256
    )


def _prep_inputs(Z_q, queue, centroids):
    """Host-side shard prep: x16 scale + e4m3 quantization + transpose to
    [C, rows], then partition-major chunk layouts so each DMA is a flat
    [128, N].  Stationary operands are pre-interleaved for SwInterleave."""
    zqT8 = _to_fp8(Z_q).T                            # [512, 256]
    zqT = zqT8.reshape(KSUB, 128, B).transpose(1, 0, 2)  # [128, KSUB, B]
    # part-2 stationary: [128, kp*2+bt, 256] interleaved
    zz = zqT8.reshape(KPAIR, 2, 128, 2, 128)         # [kp, i, p, bt, col]
    zqTi = (
        _interleave(zz[:, 0], zz[:, 1])
        .transpose(1, 0, 2, 3)
        .reshape(128, KSUB, 256)
    )
    zq = np.ascontiguousarray(np.concatenate([zqT, zqTi], axis=1))  # [128, 8, 256]

    qT = np.ascontiguousarray(_to_fp8(queue).T)      # [512, 65536]
    cT = np.ascontiguousarray(_to_fp8(centroids).T)  # [512, 50000]

    in_maps = []
    for i in range(NCORES):
        q_sh = qT[:, i * QSH : (i + 1) * QSH]        # [512, 8192]
        # [kp, i, p, h, jl, col]
        qq = q_sh.reshape(KPAIR, 2, 128, QCHUNK, JSUB, 128)
        q_sh = np.ascontiguousarray(
            _interleave(qq[:, 0], qq[:, 1]).transpose(2, 1, 3, 0, 4)
        )  # [QCHUNK, 128, JSUB, KPAIR, 256]
        qt_chunks = {f"qt{h}": q_sh[h] for h in range(QCHUNK)}
        c_sh = np.zeros((C, CSH_PAD), ml_dtypes.float8_e4m3fn)
        c_sh[:, :CSH] = cT[:, i * CSH : (i + 1) * CSH]
        # cTb0/cTb1 = first 2 matmul chunks (small, land first); cTa = rest
        def cpack(lo, hi):
            return np.ascontiguousarray(
                c_sh[:, lo * CW : hi * CW]
                .reshape(KSUB, 128, (hi - lo) * CW)
                .transpose(1, 0, 2)
            )

        head = np.ascontiguousarray(
            np.concatenate(
                [cpack(0, 1).reshape(128, -1), cpack(1, 2).reshape(128, -1),
                 cpack(2, 4).reshape(128, -1)],
                axis=1,
            )
        )  # [128, 7168]: cTb0 | cTb1 | cTa0a per partition
        in_maps.append({"zq": zq, **qt_chunks, "head": head,
                        "cTa0b": cpack(4, 6),
                        "cTa1": cpack(6, 10), "cTa2": cpack(10, 14)})
    return in_maps


def kernel(Z_q, Z_k, queue, centroids, kmeans_temp, neg_raw):
    global last_exec_time_ns
    from concourse.bass_utils import run_bass_kernel_spmd

    nc = _get_nc()
    in_maps = _prep_inputs(Z_q, queue, centroids)

    trace = bool(int(os.environ.get("MOCO_BASS_TRACE", "0")))
    out = run_bass_kernel_spmd(nc, in_maps, core_ids=list(range(NCORES)), trace=trace)
    last_exec_time_ns = out.exec_time_ns
    res = out.results

    # decode the merged export per core
    def regions(r):
        raw = np.concatenate([r["outA"], r["outB"], r["outC"], r["outD"]], axis=1)
        s2 = raw[:, :S2_BYTES].astype(np.float32).reshape(128, CCH, 2, CW)
        rm = np.ascontiguousarray(raw[:, RM_OFF:FIN_OFF]).view(np.float32)
        fin = np.ascontiguousarray(raw[:, FIN_OFF:]).view(np.float32)
        return s2, rm, fin

    decoded = [regions(r) for r in res]

    # ---- host combine (tiny) ----
    lp = (Z_q.astype(np.float64) * Z_k.astype(np.float64)).sum(axis=1)  # l_pos
    lp_t = lp / INFO_TEMP

    # part-1 loss: logsumexp over [l_pos | l_neg]/T per batch row.
    # Device partials are unshifted sums of e^{s/T} (|s/T| <= ~4).
    S = np.zeros(B, np.float64)
    for _, _, fin in decoded:
        S += fin[0].astype(np.float64)
    S += np.exp(lp_t)
    lse1 = np.log(S)
    loss1 = np.mean(lse1 - lp_t)

    # accuracy: exact despite fp8 scores.  The device reduces the fp8 exp
    # tiles over the batch axis (rm = max_b exp(s/T), fp32-exact); every
    # row with margin < MARGIN is re-checked on the host in full precision.
    rm_full = np.empty(QUEUE, np.float64)
    for i, (_, rm, _) in enumerate(decoded):
        # rm[p, jt] -> queue row j = jt*128 + p
        rm_full[i * QSH : (i + 1) * QSH] = (
            np.log(rm.astype(np.float64).T.reshape(-1)) * INFO_TEMP
        )

    # s0 computed exactly on the host (33 MFLOP) -- only rm comes from
    # the device, so the margin test has one noisy side instead of two
    s0_full = queue.astype(np.float64) @ Z_q[0].astype(np.float64)
    cand = (rm_full - s0_full) < MARGIN
    cols = np.nonzero(cand)[0]
    sub = Z_q.astype(np.float64) @ queue[cols].astype(np.float64).T  # [B, ncand]
    count = float((sub[0] >= sub.max(axis=0)).sum())
    count += float(lp[0] >= lp.max())
    accuracy = count / (1 + QUEUE)

    # part-2: global argmax over centroids (== argmin of ||c||^2 - 2 s).
    # s2 arrives in fp8; the argmax (and the positive logit) is resolved
    # exactly by re-checking every near-max column in full precision.
    s2_full = np.empty((B, NCL), np.float32)
    for i, (s2, _, _) in enumerate(decoded):
        sh = s2.transpose(2, 0, 1, 3).reshape(B, CSH_PAD)
        s2_full[:, i * CSH : (i + 1) * CSH] = sh[:, :CSH]
    s2_full /= DOT_SCALE

    kt = kmeans_temp.astype(np.float64)
    Zq64 = Z_q.astype(np.float64)
    ce64 = centroids.astype(np.float64)
    mx = s2_full.max(axis=1)
    I = np.empty(B, np.int64)
    pl_pos = np.empty(B)
    for b in range(B):
        cnd = np.nonzero(s2_full[b] >= mx[b] - MARGIN2)[0]
        ex = ce64[cnd] @ Zq64[b]
        k = int(np.argmax(ex))
        I[b] = cnd[k]
        pl_pos[b] = ex[k] / kt[cnd[k]]

    neg_idx = neg_raw + (neg_raw >= I[:, None]).astype(neg_raw.dtype)
    pl_neg = (
        np.take_along_axis(s2_full, neg_idx, axis=1).astype(np.float64)
        / kt[neg_idx]
    )
    plogits = np.concatenate([pl_pos[:, None], pl_neg], axis=1)
    m = plogits.max(axis=1)
    plse = np.log(np.exp(plogits - m[:, None]).sum(axis=1)) + m
    ploss = np.mean(plse - pl_pos)

    loss = loss1 + PROTO_FACTOR * ploss
    return np.float32(loss), np.float32(accuracy)


# revision 49
# speedup vs baseline: 1.0589x; 1.0017x over previous
"""MoCo loss (InfoNCE over a 65536-entry queue + proto-NCE over 50000
k-means centroids) on 8 Trainium2 NeuronCores.

fp8e4m3 operands with DoubleRowSwInterleave matmuls (2 contraction
subtiles per instruction; the stationary operand is pre-interleaved on
the host -- A/B k-layer pairs per column, columns reversed -- so the
weight load reads contiguously).  Tables are scaled by 16 per operand
(dots come out x256) and quantized to e4m3 on the host.

Per core (tables sharded by row, Z_q replicated):

  part 2 (centroid shard, zq stationary / centroids moving):
    - s2 = Z_q @ c_shard.T  (PE, fp32 acc), cast fp8 (DVE/ACT), export
      (argmax + exclusion gather on the host)
  part 1 (queue shard, queue stationary / zq moving):
    - s1 = q_shard @ Z_q.T                 (PE)
    - exp(s1/T) in fp8  (no shift; values in [e^-4, e^4])   (ACT)
    - per-queue-row max of the exp tiles -> rm export       (DVE)
    - ones DoubleRow matmul partition sum -> per-batch partial
      sum-of-exp                                            (PE)

The kernel streams ~7.3 MiB of table reads per core on the sync HWDGE
ring as ONE FIFO in exact consumption order (sync has strict priority
over the scalar ring, so a single queue gives both full bandwidth and
deterministic arrival order).  A ~3.4 us burst of dummy matmuls on zq
warms the PE HAM clock gate before the first real chunk lands.  Part-2
chunks bracket the part-1 backbone (head chunks early, the rest after
the last tile); exports are staged contiguous DRAM tensors -- outA/outB
mid-kernel on gpsimd, rm+fin as soon as they settle, and a short final
chunk export -- so almost nothing trails the compute.

The host combines per-core partials (logsumexp merge, global argmax,
exclusion gather + 513-wide softmax).  Both outputs stay exact despite
fp8 noise: the host computes s0 = queue @ Z_q[0] itself and re-checks
every queue row within MARGIN of the device rowmax -- and every
centroid column within MARGIN2 of its row max -- with full-precision
dot products (fp8 dot error is < 0.01).
"""

import os
import numpy as np
import ml_dtypes

B, C = 256, 512
QUEUE, NCL, NNEG = 65536, 50000, 512
INFO_TEMP = 0.07
PROTO_FACTOR = 0.5
NCORES = 8
QSH = QUEUE // NCORES          # 8192 queue rows per core
CSH = NCL // NCORES            # 6250 centroid rows per core
CSH_PAD = 6272                 # 14 * 448
CCH = 14                       # s2 matmul chunks
CW = CSH_PAD // CCH            # 448
KSUB = C // 128                # 4 contraction subtiles
KPAIR = KSUB // 2              # 2 DoubleRow pairs
NBT = 16                       # part-1 big tiles (512 queue rows each)
QCHUNK = 4                     # qT DMA chunks (8 KiB per-partition lines)
JW = QSH // QCHUNK             # 2048
JSUB = JW // 128               # 16 queue subtiles per chunk
FP8_SCALE = 16.0               # per-operand scale; dots come out x256
DOT_SCALE = FP8_SCALE * FP8_SCALE
MARGIN = 0.05                  # host re-check threshold (unscaled units)
MARGIN2 = 0.045                # part-2 argmax re-check threshold

# merged export regions (bytes per partition inside the SBUF out tile).
# DRAM side is THREE separate contiguous tensors (outA/outB/outC) --
# strided DRAM writes run ~4x slower than contiguous ones.
S2_BYTES = CCH * 2 * CW        # 12544
RM_OFF = S2_BYTES              # [128, NBT*4] fp32 = 256 B
FIN_OFF = RM_OFF + NBT * 4 * 4  # [128, 256] fp32 = 1024 B
OUT_BYTES = FIN_OFF + B * 4    # 13824
OUTA_END = 6 * 2 * CW          # chunks 0-5   (5376 B lines)
OUTB_END = 10 * 2 * CW         # chunks 6-9   (3584 B lines)
# outC = chunks 10-13 + rm + fin (4864 B lines, the final export)

# s2 cast engines: head chunks + late-tail evens on DVE, the rest on ACT
# (ACT covers the early tail while DVE finishes the rm reduces)
DVE_CAST_CH = frozenset((0, 1, 2, 3, 4, 5, 10, 12))
# rm reduce groups: big groups while exps stream; the last 4 tiles reduce
# per half-tile so the post-exp tail chain is one 512-elem reduce
RM_GROUPS = ((0, 4), (4, 8), (8, 10), (10, 12))
RM_HALF_FROM = 12

_CACHE = {}

# exec time of the last device run (ns), populated when tracing is on
last_exec_time_ns = None


def _build():
    import concourse.bass as bass
    import concourse.tile as tile
    from concourse import bacc, mybir

    dt = mybir.dt
    DRI = mybir.MatmulPerfMode.DoubleRowSwInterleave
    nc = bacc.Bacc(
        "TRN2", target_bir_lowering=False, debug=False, num_devices=NCORES
    )

    # ---- DRAM I/O (all partition-major so every DMA is [128, N] flat) ----
    zq_d = nc.dram_tensor("zq", [128, 8, 256], dt.float8e4, kind="ExternalInput").ap()
    qt_d = [
        nc.dram_tensor(
            f"qt{h}", [128, JSUB, KPAIR, 256], dt.float8e4, kind="ExternalInput"
        ).ap()
        for h in range(QCHUNK)
    ]
    # head = cTb0 | cTb1 | cTa0a in one transfer (lands in one burst)
    head_d = nc.dram_tensor(
        "head", [128, 2 * KSUB * CW + KSUB * 2 * CW], dt.float8e4,
        kind="ExternalInput",
    ).ap()
    cTa0b_d = nc.dram_tensor(
        "cTa0b", [128, KSUB, 2 * CW], dt.float8e4, kind="ExternalInput"
    ).ap()
    cTa1_d = nc.dram_tensor(
        "cTa1", [128, KSUB, 4 * CW], dt.float8e4, kind="ExternalInput"
    ).ap()
    cTa2_d = nc.dram_tensor(
        "cTa2", [128, KSUB, 4 * CW], dt.float8e4, kind="ExternalInput"
    ).ap()

    # merged output, three contiguous DRAM tensors:
    # outA = s2 chunks 0-5, outB = chunks 6-11, outC = chunks 12-13|rm|fin
    outA_d = nc.dram_tensor(
        "outA", [128, OUTA_END], dt.float8e4, kind="ExternalOutput"
    ).ap()
    outB_d = nc.dram_tensor(
        "outB", [128, OUTB_END - OUTA_END], dt.float8e4, kind="ExternalOutput"
    ).ap()
    outC_d = nc.dram_tensor(
        "outC", [128, S2_BYTES - OUTB_END], dt.float8e4, kind="ExternalOutput"
    ).ap()
    outD_d = nc.dram_tensor(
        "outD", [128, OUT_BYTES - S2_BYTES], dt.float8e4, kind="ExternalOutput"
    ).ap()

    with tile.TileContext(nc) as tc:
        with (
            tc.tile_pool(name="const", bufs=1) as cpool,
            tc.tile_pool(name="ps1", bufs=2, space="PSUM") as ps1,
            tc.tile_pool(name="psum1s", bufs=1, space="PSUM") as ps1s,
            tc.tile_pool(name="ps2", bufs=3, space="PSUM") as ps2,
        ):
            # ---- resident SBUF tensors ----
            zq_sb = cpool.tile([128, 8, 256], dt.float8e4)
            head_sb = cpool.tile([128, 4 * KSUB * CW], dt.float8e4, tag="head")
            cTb_sb = [
                head_sb[:, i * KSUB * CW : (i + 1) * KSUB * CW].rearrange(
                    "p (k w) -> p k w", k=KSUB, w=CW
                )
                for i in range(2)
            ]
            cTa0a_sb = head_sb[:, 2 * KSUB * CW :].rearrange(
                "p (k w) -> p k w", k=KSUB, w=2 * CW
            )
            cTa0b_sb = cpool.tile([128, KSUB, 2 * CW], dt.float8e4, tag="cTa0b")
            cTa1_sb = cpool.tile([128, KSUB, 4 * CW], dt.float8e4, tag="cTa1")
            cTa2_sb = cpool.tile([128, KSUB, 4 * CW], dt.float8e4, tag="cTa2")
            qt_sb = [
                cpool.tile(
                    [128, JSUB, KPAIR, 256], dt.float8e4, name=f"qt{h}", tag=f"qt{h}"
                )
                for h in range(QCHUNK)
            ]

            # ALL input DMAs on the sync HWDGE ring: it has strict priority
            # over the scalar ring, so one FIFO in exact consumption order
            # gives full bandwidth to the critical stream and exact arrival
            # order.  Exports ride gpsimd (SWDGE) except the final outC.
            nc.sync.dma_start(zq_sb[:], zq_d[:])
            nc.sync.dma_start(head_sb[:], head_d[:])
            nc.sync.dma_start(qt_sb[0][:], qt_d[0][:])
            nc.sync.dma_start(cTa0b_sb[:], cTa0b_d[:])
            nc.sync.dma_start(qt_sb[1][:], qt_d[1][:])
            nc.sync.dma_start(qt_sb[2][:], qt_d[2][:])
            nc.sync.dma_start(qt_sb[3][:], qt_d[3][:])
            nc.sync.dma_start(cTa1_sb[:], cTa1_d[:])
            nc.sync.dma_start(cTa2_sb[:], cTa2_d[:])

            # interleave/column-reversal of all-ones is all-ones
            ones_sb = cpool.tile([128, 256], dt.float8e4)
            nc.vector.memset(ones_sb[:], 1.0)

            # merged export tile + typed views
            out_sb = cpool.tile([128, OUT_BYTES], dt.float8e4)
            s2_v = out_sb[:, 0:S2_BYTES].rearrange(
                "p (c b w) -> p c b w", c=CCH, b=2, w=CW
            )
            rm_v = out_sb[:, RM_OFF:FIN_OFF].bitcast(dt.float32)   # [128, 64]
            fin_v = out_sb[:, FIN_OFF:OUT_BYTES].bitcast(dt.float32)  # [128, 256]

            exp_all = cpool.tile([128, NBT, 4, B], dt.float8e4)
            exp_tiles = [exp_all[:, t] for t in range(NBT)]

            fin_sb = None  # ACT writes fin via fin_v

            # ---- emission helpers ----
            def emit_chunk(ch):
                """part-2 chunk: s2[:, ch] = Z_q @ c_chunk.T, cast to fp8."""
                if ch < 2:
                    cmov, w = cTb_sb[ch], 0
                elif ch < 4:
                    cmov, w = cTa0a_sb, ch - 2
                elif ch < 6:
                    cmov, w = cTa0b_sb, ch - 4
                elif ch < 10:
                    cmov, w = cTa1_sb, ch - 6
                else:
                    cmov, w = cTa2_sb, ch - 10
                for bt in range(2):
                    s2_ps = ps2.tile([128, CW], dt.float32, tag="s2")
                    for kp in range(KPAIR):
                        nc.tensor.matmul(
                            s2_ps[:],
                            zq_sb[:, 4 + 2 * kp + bt, :],
                            cmov[:, 2 * kp : 2 * kp + 2, w * CW : (w + 1) * CW],
                            start=(kp == 0),
                            stop=(kp == KPAIR - 1),
                            perf_mode=DRI,
                        )
                    if ch == CCH - 1:
                        # the very last chunk: split its two casts across
                        # both engines so the final export issues sooner
                        eng = nc.vector.tensor_copy if bt == 0 else nc.scalar.copy
                        eng(s2_v[:, ch, bt, :], s2_ps[:])
                    elif ch in DVE_CAST_CH:
                        nc.vector.tensor_copy(s2_v[:, ch, bt, :], s2_ps[:])
                    else:
                        nc.scalar.copy(s2_v[:, ch, bt, :], s2_ps[:])

            rm_after = {b - 1: (a, b) for a, b in RM_GROUPS}

            def emit_tile(t, p1s_ps):
                """part-1 tile: 512 queue rows -> exp fp8; lagged ones-sum;
                rm reduce at group boundaries (per half-tile near the end)."""
                s1_ps = ps1.tile([128, 4, B], dt.float32, tag="s1")
                for q in range(4):
                    jt = t * 4 + q
                    h, jl = divmod(jt, JSUB)
                    for kp in range(KPAIR):
                        nc.tensor.matmul(
                            s1_ps[:, q, :],
                            qt_sb[h][:, jl, kp, :],
                            zq_sb[:, 2 * kp : 2 * kp + 2, :],
                            start=(kp == 0),
                            stop=(kp == KPAIR - 1),
                            perf_mode=DRI,
                        )
                nc.scalar.activation(
                    exp_tiles[t][:],
                    s1_ps[:],
                    mybir.ActivationFunctionType.Exp,
                    scale=1.0 / (DOT_SCALE * INFO_TEMP),
                )
                if t > 1:
                    # two tiles behind: ACT's exp has a full tile of slack
                    for g in range(2):
                        nc.tensor.matmul(
                            p1s_ps[:],
                            ones_sb[:],
                            exp_tiles[t - 2][:, 2 * g : 2 * g + 2, :],
                            start=(t == 2 and g == 0),
                            stop=False,
                            perf_mode=DRI,
                        )
                if t >= RM_HALF_FROM:
                    for hh in range(2):
                        nc.vector.tensor_reduce(
                            rm_v[:, t * 4 + 2 * hh : t * 4 + 2 * hh + 2],
                            exp_all[:, t, 2 * hh : 2 * hh + 2],
                            axis=mybir.AxisListType.X,
                            op=mybir.AluOpType.max,
                        )
                elif t in rm_after:
                    a, b = rm_after[t]
                    nc.vector.tensor_reduce(
                        rm_v[:, a * 4 : b * 4],
                        exp_all[:, a:b],
                        axis=mybir.AxisListType.X,
                        op=mybir.AluOpType.max,
                    )

            # ---- emission: warmup, head chunks, part-1 backbone, tail ----
            p1s_ps = ps1s.tile([128, B], dt.float32)   # sum-of-exp accum

            # HAM warmup: ~3.4us of dummy matmuls on zq (the first input to
            # land) so the real work runs at 2.4 GHz from the start.  The
            # results are never read; the psum pool recycles the banks.
            for i in range(16):
                w_ps = ps2.tile([128, CW], dt.float32, tag="s2")
                nc.tensor.matmul(
                    w_ps[:, 0:B],
                    zq_sb[:, 4, :],
                    zq_sb[:, 0:2, :],
                    start=True,
                    stop=True,
                    perf_mode=DRI,
                )

            for ch in range(4):
                emit_chunk(ch)
            for t in range(4):
                emit_tile(t, p1s_ps)
            emit_chunk(4)
            emit_chunk(5)
            for t in range(4, NBT):
                emit_tile(t, p1s_ps)
            # outA: chunks 0-5 (overlaps the remaining compute)
            nc.gpsimd.dma_start(outA_d[:], out_sb[:, 0:OUTA_END])
            # chunks 6-7 keep the PE busy while ACT finishes exp t14/t15
            # (the lagged ones-flush below waits on those exps)
            emit_chunk(6)
            emit_chunk(7)
            # flush the lagged ones-sum (tiles 14, 15)
            for t in (NBT - 2, NBT - 1):
                for g in range(2):
                    nc.tensor.matmul(
                        p1s_ps[:],
                        ones_sb[:],
                        exp_tiles[t][:, 2 * g : 2 * g + 2, :],
                        start=False,
                        stop=(t == NBT - 1 and g == 1),
                        perf_mode=DRI,
                    )
            nc.scalar.copy(fin_v[:], p1s_ps[:])
            # rm + fin are ready well before the last chunks: export now
            nc.sync.dma_start(outD_d[:], out_sb[:, S2_BYTES:OUT_BYTES])
            emit_chunk(8)
            emit_chunk(9)
            nc.gpsimd.dma_start(outB_d[:], out_sb[:, OUTA_END:OUTB_END])
            for ch in range(10, CCH):
                emit_chunk(ch)
            # final export: chunks 10-13 (3584 B lines)
            nc.sync.dma_start(outC_d[:], out_sb[:, OUTB_END:S2_BYTES])

    nc.compile()
    return nc


def _get_nc():
    if "nc" not in _CACHE:
        _CACHE["nc"] = _build()
    return _CACHE["nc"]


def _to_fp8(x):
    return (x * FP8_SCALE).astype(ml_dtypes.float8_e4m3fn)


def _interleave(A, B):
    """SwInterleave weight layout: mem[p, 2*jj+i] = layer_i[p, 127-jj].
    A, B: [..., 128, 128] (partition, column)."""
    return np.stack([A[..., ::-1], B[..., ::-1]], axis=-1).reshape(
        *A.shape[:-1], 256
    )


def _prep_inputs(Z_q, queue, centroids):
    """Host-side shard prep: x16 scale + e4m3 quantization + transpose to
    [C, rows], then partition-major chunk layouts so each DMA is a flat
    [128, N].  Stationary operands are pre-interleaved for SwInterleave."""
    zqT8 = _to_fp8(Z_q).T                            # [512, 256]
    zqT = zqT8.reshape(KSUB, 128, B).transpose(1, 0, 2)  # [128, KSUB, B]
    # part-2 stationary: [128, kp*2+bt, 256] interleaved
    zz = zqT8.reshape(KPAIR, 2, 128, 2, 128)         # [kp, i, p, bt, col]
    zqTi = (
        _interleave(zz[:, 0], zz[:, 1])
        .transpose(1, 0, 2, 3)
        .reshape(128, KSUB, 256)
    )
    zq = np.ascontiguousarray(np.concatenate([zqT, zqTi], axis=1))  # [128, 8, 256]

    qT = np.ascontiguousarray(_to_fp8(queue).T)      # [512, 65536]
    cT = np.ascontiguousarray(_to_fp8(centroids).T)  # [512, 50000]

    in_maps = []
    for i in range(NCORES):
        q_sh = qT[:, i * QSH : (i + 1) * QSH]        # [512, 8192]
        # [kp, i, p, h, jl, col]
        qq = q_sh.reshape(KPAIR, 2, 128, QCHUNK, JSUB, 128)
        q_sh = np.ascontiguousarray(
            _interleave(qq[:, 0], qq[:, 1]).transpose(2, 1, 3, 0, 4)
        )  # [QCHUNK, 128, JSUB, KPAIR, 256]
        qt_chunks = {f"qt{h}": q_sh[h] for h in range(QCHUNK)}
        c_sh = np.zeros((C, CSH_PAD), ml_dtypes.float8_e4m3fn)
        c_sh[:, :CSH] = cT[:, i * CSH : (i + 1) * CSH]
        # cTb0/cTb1 = first 2 matmul chunks (small, land first); cTa = rest
        def cpack(lo, hi):
            return np.ascontiguousarray(
                c_sh[:, lo * CW : hi * CW]
                .reshape(KSUB, 128, (hi - lo) * CW)
                .transpose(1, 0, 2)
            )

        head = np.ascontiguousarray(
            np.concatenate(
                [cpack(0, 1).reshape(128, -1), cpack(1, 2).reshape(128, -1),
                 cpack(2, 4).reshape(128, -1)],
                axis=1,
            )
        )  # [128, 7168]: cTb0 | cTb1 | cTa0a per partition
        in_maps.append({"zq": zq, **qt_chunks, "head": head,
                        "cTa0b": cpack(4, 6),
                        "cTa1": cpack(6, 10), "cTa2": cpack(10, 14)})
    return in_maps


def kernel(Z_q, Z_k, queue, centroids, kmeans_temp, neg_raw):
    global last_exec_time_ns
    from concourse.bass_utils import run_bass_kernel_spmd

    nc = _get_nc()
    in_maps = _prep_inputs(Z_q, queue, centroids)

    trace = bool(int(os.environ.get("MOCO_BASS_TRACE", "0")))
    out = run_bass_kernel_spmd(nc, in_maps, core_ids=list(range(NCORES)), trace=trace)
    last_exec_time_ns = out.exec_time_ns
    res = out.results

    # decode the merged export per core
    def regions(r):
        raw = np.concatenate([r["outA"], r["outB"], r["outC"], r["outD"]], axis=1)
        s2 = raw[:, :S2_BYTES].astype(np.float32).reshape(128, CCH, 2, CW)
        rm = np.ascontiguousarray(raw[:, RM_OFF:FIN_OFF]).view(np.float32)
        fin = np.ascontiguousarray(raw[:, FIN_OFF:]).view(np.float32)
        return s2, rm, fin

    decoded = [regions(r) for r in res]

    # ---- host combine (tiny) ----
    lp = (Z_q.astype(np.float64) * Z_k.astype(np.float64)).sum(axis=1)  # l_pos
    lp_t = lp / INFO_TEMP

    # part-1 loss: logsumexp over [l_pos | l_neg]/T per batch row.
    # Device partials are unshifted sums of e^{s/T} (|s/T| <= ~4).
    S = np.zeros(B, np.float64)
    for _, _, fin in decoded:
        S += fin[0].astype(np.float64)
    S += np.exp(lp_t)
    lse1 = np.log(S)
    loss1 = np.mean(lse1 - lp_t)

    # accuracy: exact despite fp8 scores.  The device reduces the fp8 exp
    # tiles over the batch axis (rm = max_b exp(s/T), fp32-exact); every
    # row with margin < MARGIN is re-checked on the host in full precision.
    rm_full = np.empty(QUEUE, np.float64)
    for i, (_, rm, _) in enumerate(decoded):
        # rm[p, jt] -> queue row j = jt*128 + p
        rm_full[i * QSH : (i + 1) * QSH] = (
            np.log(rm.astype(np.float64).T.reshape(-1)) * INFO_TEMP
        )

    # s0 computed exactly on the host (33 MFLOP) -- only rm comes from
    # the device, so the margin test has one noisy side instead of two
    s0_full = queue.astype(np.float64) @ Z_q[0].astype(np.float64)
    cand = (rm_full - s0_full) < MARGIN
    cols = np.nonzero(cand)[0]
    sub = Z_q.astype(np.float64) @ queue[cols].astype(np.float64).T  # [B, ncand]
    count = float((sub[0] >= sub.max(axis=0)).sum())
    count += float(lp[0] >= lp.max())
    accuracy = count / (1 + QUEUE)

    # part-2: global argmax over centroids (== argmin of ||c||^2 - 2 s).
    # s2 arrives in fp8; the argmax (and the positive logit) is resolved
    # exactly by re-checking every near-max column in full precision.
    s2_full = np.empty((B, NCL), np.float32)
    for i, (s2, _, _) in enumerate(decoded):
        sh = s2.transpose(2, 0, 1, 3).reshape(B, CSH_PAD)
        s2_full[:, i * CSH : (i + 1) * CSH] = sh[:, :CSH]
    s2_full /= DOT_SCALE

    kt = kmeans_temp.astype(np.float64)
    Zq64 = Z_q.astype(np.float64)
    ce64 = centroids.astype(np.float64)
    mx = s2_full.max(axis=1)
    I = np.empty(B, np.int64)
    pl_pos = np.empty(B)
    for b in range(B):
        cnd = np.nonzero(s2_full[b] >= mx[b] - MARGIN2)[0]
        ex = ce64[cnd] @ Zq64[b]
        k = int(np.argmax(ex))
        I[b] = cnd[k]
        pl_pos[b] = ex[k] / kt[cnd[k]]

    neg_idx = neg_raw + (neg_raw >= I[:, None]).astype(neg_raw.dtype)
    pl_neg = (
        np.take_along_axis(s2_full, neg_idx, axis=1).astype(np.float64)
        / kt[neg_idx]
    )
    plogits = np.concatenate([pl_pos[:, None], pl_neg], axis=1)
    m = plogits.max(axis=1)
    plse = np.log(np.exp(plogits - m[:, None]).sum(axis=1)) + m
    ploss = np.mean(plse - pl_pos)

    loss = loss1 + PROTO_FACTOR * ploss
    return np.float32(loss), np.float32(accuracy)
